# revision 1
# baseline (speedup 1.0000x reference)
"""Distributed 2-layer GCN (BangaloreGCN) on 8 Trainium2 NeuronCores.

Strategy (node/graph-parallel, per spec sharding hint):
  * Nodes are packed into 8*49 destination tiles of 128 slots (LPT on
    in-degree so every tile's incoming-edge count fits a fixed chunk
    budget -> fully static SPMD program).
  * GCN algebra is refactored so message passing is a pure gather +
    segment-sum:  out = dinv * (A @ (dinv*h)) + dinv^2 * h, with the
    per-channel BN scale folded into W, biases folded into a post-add.
  * Per layer: each core computes its shard of the (dinv*h) table,
    AllGather replicates it to HBM on every core, then each core
    dma_gathers the source rows for its own edges and segment-sums them
    with one-hot selection matmuls into PSUM (128 dests x 64 feats).
  * int16 gather indices only span 32768 rows, so edges are split into
    a "low" pass (table rows [0, 32768)) and "high" pass (rows
    [NSLOT-32768, NSLOT)); edges in the overlap are assigned to balance
    per-tile chunk counts.
"""

import sys

sys.path.insert(0, "/opt/trn_rl_repo")

import heapq

import ml_dtypes
import numpy as np

BF16 = ml_dtypes.bfloat16

# ---- problem constants (hardcoded per contest contract) ----
N_NODES = 50000
IN_CH = 128
HID = 64
HID2 = 32
BN_EPS = 1e-5

NCORES = 8
P = 128
TILES = 49                 # dest tiles per core
SPC = TILES * P            # slots per core (6272)
NSLOT = NCORES * SPC       # 50176
NBINS = NCORES * TILES
LO_LIM = 32768             # low gather table covers rows [0, 32768)
HI_BASE = NSLOT - 32768    # high table covers [HI_BASE, NSLOT)
GT = 7                     # dest tiles per dma_gather call
NCALLS = TILES // GT
PAD_DEST = 200.0
TBW = 128                  # padded table row width (bf16 -> 256B elems)

USE_BF16 = True


# ----------------------------------------------------------------------
# host-side preparation
# ----------------------------------------------------------------------
def _pack_nodes(deg_in, n):
    order = np.argsort(-deg_in, kind="stable")
    heap = [(0, b) for b in range(NBINS)]
    heapq.heapify(heap)
    counts = np.zeros(NBINS, np.int32)
    binof = np.empty(n, np.int32)
    for v in order:
        load, b = heapq.heappop(heap)
        binof[v] = b
        counts[b] += 1
        if counts[b] < P:
            heapq.heappush(heap, (load + int(deg_in[v]), b))
    perm = np.argsort(binof, kind="stable")
    ptr = np.zeros(NBINS, np.int32)
    lanes = np.empty(n, np.int32)
    for v in perm:
        b = binof[v]
        lanes[v] = ptr[b]
        ptr[b] += 1
    return binof.astype(np.int64) * P + lanes


def _wrap_idx(arr):
    ni = arr.shape[0]
    blk = arr.reshape(ni // 16, 16).T.astype(np.int16)
    return np.tile(blk, (8, 1))


def host_prep(x, edge_index, W1, b1, W2, b2, fcW, fcb,
              g1, be1, rm1, rv1, g2, be2, rm2, rv2):
    n = x.shape[0]
    row = np.asarray(edge_index[0], np.int64)
    col = np.asarray(edge_index[1], np.int64)

    deg = np.bincount(col, minlength=n).astype(np.float32) + 1.0
    dinv = (1.0 / np.sqrt(deg)).astype(np.float32)
    deg_in = np.bincount(col, minlength=n)

    slot_of_node = _pack_nodes(deg_in, n)
    node_of_slot = np.full(NSLOT, -1, np.int64)
    node_of_slot[slot_of_node] = np.arange(n)

    src_slot = slot_of_node[row]
    dst_slot = slot_of_node[col]
    dbin = dst_slot // P
    dlane = dst_slot % P

    order = np.argsort(dbin, kind="stable")
    src_s = src_slot[order]
    dlane_s = dlane[order]
    dbin_s = dbin[order]
    starts = np.searchsorted(dbin_s, np.arange(NBINS))
    ends = np.searchsorted(dbin_s, np.arange(NBINS) + 1)

    nA_min = np.zeros(NBINS, np.int64)
    nB_min = np.zeros(NBINS, np.int64)
    tot = ends - starts
    for b in range(NBINS):
        s = src_s[starts[b]:ends[b]]
        nA_min[b] = int((s < HI_BASE).sum())
        nB_min[b] = int((s >= LO_LIM).sum())
    maxA, maxB, maxT = int(nA_min.max()), int(nB_min.max()), int(tot.max())
    best = None
    for ct in range(-(-maxT // P), -(-maxT // P) + 8):
        for ca in range(-(-maxA // P), ct + 1):
            cb = ct - ca
            if cb >= 0 and cb * P >= maxB:
                best = (ca, cb)
                break
        if best:
            break
    CA, CB = best
    capA, capB = CA * P, CB * P

    srcA = np.zeros((NBINS, capA), np.int64)
    destA = np.full((NBINS, capA), PAD_DEST, np.float32)
    srcB = np.zeros((NBINS, capB), np.int64)
    destB = np.full((NBINS, capB), PAD_DEST, np.float32)
    for b in range(NBINS):
        s = src_s[starts[b]:ends[b]]
        d = dlane_s[starts[b]:ends[b]]
        isB_must = s >= LO_LIM
        isA_must = s < HI_BASE
        mid_idx = np.where(~isB_must & ~isA_must)[0]
        room = capB - int(isB_must.sum())
        takeB = mid_idx[:room]
        selB = np.concatenate([np.where(isB_must)[0], takeB])
        selA = np.concatenate([np.where(isA_must)[0], mid_idx[room:]])
        assert len(selB) <= capB and len(selA) <= capA
        srcB[b, :len(selB)] = s[selB] - HI_BASE
        destB[b, :len(selB)] = d[selB]
        srcA[b, :len(selA)] = s[selA]
        destA[b, :len(selA)] = d[selA]

    S1c = (g1 / np.sqrt(rv1 + BN_EPS)).astype(np.float32)
    T1 = ((b1 - rm1) * S1c + be1).astype(np.float32)
    S2c = (g2 / np.sqrt(rv2 + BN_EPS)).astype(np.float32)
    T2 = ((b2 - rm2) * S2c + be2).astype(np.float32)
    W1p = (W1 * S1c[None, :]).astype(np.float32)
    W2p = (W2 * S2c[None, :]).astype(np.float32)

    NCH = CA + CB
    cores = []
    for c in range(NCORES):
        tsl = slice(c * TILES, (c + 1) * TILES)
        sA = srcA[tsl].reshape(-1)
        sB = srcB[tsl].reshape(-1)
        idxA_img = np.hstack(
            [_wrap_idx(sA[g * GT * capA:(g + 1) * GT * capA]) for g in range(NCALLS)])
        idxB_img = np.hstack(
            [_wrap_idx(sB[g * GT * capB:(g + 1) * GT * capB]) for g in range(NCALLS)])
        dst_img = np.zeros((P, TILES * NCH), np.float32)
        for tl in range(TILES):
            b = c * TILES + tl
            dst_img[:, tl * NCH:tl * NCH + CA] = destA[b].reshape(CA, P).T
            dst_img[:, tl * NCH + CA:(tl + 1) * NCH] = destB[b].reshape(CB, P).T
        nodes = node_of_slot[c * SPC:(c + 1) * SPC]
        occ = nodes >= 0
        xs = np.zeros((SPC, IN_CH), np.float32)
        xs[occ] = x[nodes[occ]] * dinv[nodes[occ], None]
        dv = np.zeros(SPC, np.float32)
        dv[occ] = dinv[nodes[occ]]
        cores.append(dict(
            idxA=idxA_img, idxB=idxB_img,
            dest=dst_img.astype(BF16) if USE_BF16 else dst_img,
            xT=np.ascontiguousarray(xs.T),
            dinv=np.ascontiguousarray(dv.reshape(TILES, P).T),
        ))

    consts = dict(W1p=W1p, W2p=W2p, T1=T1, T2=T2,
                  fcW=np.asarray(fcW, np.float32), fcb=float(np.asarray(fcb).reshape(-1)[0]),
                  CA=CA, CB=CB, node_of_slot=node_of_slot)
    return cores, consts


# ----------------------------------------------------------------------
# device program
# ----------------------------------------------------------------------
def _dma_gather_raw(gp, bassmod, out_ap, in_ap, idxs_ap, num_idxs, elem_size,
                    elem_step, single_packet=True, queue_num=0):
    """bass.dma_gather with elem_size_bytes below 256B allowed (stride must
    still be a multiple of 256B). Verified on HW (see work/smoke4.py)."""
    import concourse.mybir as mybir
    from concourse import ap_utils
    from concourse.bass import MemorySpace, exact_div, round_up_to_multiple

    assert idxs_ap.dtype == mybir.dt.int16
    assert in_ap.dtype == out_ap.dtype
    assert in_ap.space == MemorySpace.DRAM
    assert idxs_ap.space == MemorySpace.SBUF and out_ap.space == MemorySpace.SBUF
    assert ap_utils.ap_is_contiguous(out_ap.ap[1:])
    assert ap_utils.ap_is_contiguous(idxs_ap.ap[1:])
    assert in_ap.ap[-1][1] == out_ap.ap[-1][1] == elem_size
    assert out_ap.ap[0][1] * out_ap.ap[1][1] == round_up_to_multiple(num_idxs, 128)
    assert in_ap.ap[0][0] == elem_step
    stride_bytes_256 = exact_div(elem_step * mybir.dt.size(in_ap.dtype), 256)
    assert stride_bytes_256 < 256
    return gp.add_instruction(
        mybir.InstDMAGatherAnt(
            name=bassmod.get_next_instruction_name(),
            ins=[*gp.lower_ap_dma(in_ap, for_custom_bir_dma=True),
                 gp.lower_ap(idxs_ap),
                 gp.lower_val_access(gp.to_reg(num_idxs))],
            outs=[gp.lower_ap(out_ap)],
            transpose=False,
            num_idxs=num_idxs,
            elem_size=elem_size,
            stride_bytes_256=stride_bytes_256,
            gen_mode=0,
            single_packet=single_packet,
            queue_num=queue_num,
            sbuf_tokens_per_rank=0,
            sbuf_free_dim_per_rank=0,
            sbuf_free_dim_pad_per_rank=0,
            sbuf_byte_offset=0,
        ))


def build_bass(CA, CB):
    import concourse.bacc as bacc
    import concourse.bass as bassm
    import concourse.mybir as mybir
    import concourse.tile as tile
    from concourse.library_config import mlp
    from concourse.masks import make_identity

    f32 = mybir.dt.float32
    bf = mybir.dt.bfloat16 if USE_BF16 else f32
    i16 = mybir.dt.int16
    tbw = TBW if USE_BF16 else HID
    NCH = CA + CB
    capA, capB = CA * P, CB * P
    wA = GT * capA // 16
    wB = GT * capB // 16

    nc = bacc.Bacc("TRN2", target_bir_lowering=False)
    xT_d = nc.dram_tensor("xT", [P, SPC], bf, kind="ExternalInput")
    idxA_d = nc.dram_tensor("idxA", [P, TILES * capA // 16], i16, kind="ExternalInput")
    idxB_d = nc.dram_tensor("idxB", [P, TILES * capB // 16], i16, kind="ExternalInput")
    dest_d = nc.dram_tensor("dest", [P, TILES * NCH], bf, kind="ExternalInput")
    dinv_d = nc.dram_tensor("dinv", [P, TILES], f32, kind="ExternalInput")
    w1_d = nc.dram_tensor("w1", [IN_CH, HID], bf, kind="ExternalInput")
    w2_d = nc.dram_tensor("w2", [HID, HID2], f32, kind="ExternalInput")
    t1_d = nc.dram_tensor("t1", [P, HID], f32, kind="ExternalInput")
    t2_d = nc.dram_tensor("t2", [P, HID2], f32, kind="ExternalInput")
    fcw_d = nc.dram_tensor("fcw", [P, HID2], f32, kind="ExternalInput")
    y_d = nc.dram_tensor("y", [P, TILES], f32, kind="ExternalOutput")

    with tile.TileContext(nc) as tc:
        with (
            tc.tile_pool(name="const", bufs=1) as cpool,
            tc.tile_pool(name="upart", bufs=1) as upool,
            tc.tile_pool(name="ga", bufs=3) as gapool,
            tc.tile_pool(name="gb", bufs=2) as gbpool,
            tc.tile_pool(name="sel", bufs=20) as selpool,
            tc.tile_pool(name="work", bufs=4) as wpool,
            tc.tile_pool(name="pmm", bufs=2, space="PSUM") as pmm,
            tc.tile_pool(name="pacc", bufs=3, space="PSUM") as pacc,
            tc.tile_pool(name="ptr", bufs=1, space="PSUM") as ptr,
            tc.tile_pool(name="p3", bufs=2, space="PSUM") as p3pool,
            tc.tile_pool(name="dram", bufs=1, space="DRAM") as dpool,
        ):
            nc.gpsimd.load_library(mlp)

            # ---- constants ----
            idxA_t = cpool.tile([P, TILES * capA // 16], i16)
            nc.sync.dma_start(out=idxA_t[:], in_=idxA_d[:])
            idxB_t = cpool.tile([P, TILES * capB // 16], i16)
            nc.sync.dma_start(out=idxB_t[:], in_=idxB_d[:])
            dest_t = cpool.tile([P, TILES * NCH], bf)
            nc.sync.dma_start(out=dest_t[:], in_=dest_d[:])
            dinv_t = cpool.tile([P, TILES], f32)
            nc.sync.dma_start(out=dinv_t[:], in_=dinv_d[:])
            w1_t = cpool.tile([IN_CH, HID], bf)
            nc.sync.dma_start(out=w1_t[:], in_=w1_d[:])
            w2_t = cpool.tile([HID, HID2], f32)
            nc.sync.dma_start(out=w2_t[:], in_=w2_d[:])
            t1_t = cpool.tile([P, HID], f32)
            nc.sync.dma_start(out=t1_t[:], in_=t1_d[:])
            t2_t = cpool.tile([P, HID2], f32)
            nc.sync.dma_start(out=t2_t[:], in_=t2_d[:])
            fcw_t = cpool.tile([P, HID2], f32)
            nc.sync.dma_start(out=fcw_t[:], in_=fcw_d[:])

            ident = cpool.tile([P, P], f32)
            make_identity(nc, ident[:])
            ones_row = cpool.tile([1, P], f32)
            nc.gpsimd.memset(ones_row[:], 1.0)
            iota_i = cpool.tile([P, NCH * P], mybir.dt.int32)
            nc.gpsimd.iota(iota_i[:], pattern=[[0, NCH], [1, P]], base=0,
                           channel_multiplier=0)
            iota_b = cpool.tile([P, NCH * P], bf)
            nc.vector.tensor_copy(out=iota_b[:], in_=iota_i[:])

            u1_t = upool.tile([P, TILES * HID], f32, tag="u1")
            s2_t = upool.tile([P, TILES * HID], f32, tag="s2")
            out_t = upool.tile([P, TILES], f32, tag="out")

            ag1_in = dpool.tile([SPC, tbw], bf)
            s1_tab = dpool.tile([NSLOT, tbw], bf, addr_space="Shared")
            ag2_in = dpool.tile([SPC, tbw], bf)
            s2_tab = dpool.tile([NSLOT, tbw], bf, addr_space="Shared")

            # ---- L1 dense: u = (x*dinv) @ W1' ----
            xfull = cpool.tile([P, SPC], bf)
            nc.sync.dma_start(out=xfull[:], in_=xT_d[:])
            for t in range(TILES):
                pm = pmm.tile([P, HID], f32, space="PSUM", tag="pm")
                nc.tensor.matmul(out=pm[:], lhsT=xfull[:, t * P:(t + 1) * P],
                                 rhs=w1_t[:], start=True, stop=True)
                nc.scalar.activation(out=u1_t[:, t * HID:(t + 1) * HID], in_=pm[:],
                                     func=mybir.ActivationFunctionType.Copy)

            nc.gpsimd.dma_start(
                out=ag1_in[:].rearrange("(t p) w -> p t w", p=P)[:, :, 0:HID],
                in_=u1_t[:].rearrange("p (t f) -> p t f", f=HID),
            )
            nc.gpsimd.collective_compute(
                "AllGather", mybir.AluOpType.bypass,
                replica_groups=[list(range(NCORES))],
                ins=[ag1_in[:]], outs=[s1_tab[:]],
            )

            def tab_ap(tab, lo, cnt):
                return bassm.AP(tensor=tab[:].tensor, offset=lo * tbw,
                                ap=[[tbw, cnt], [1, HID]])

            def scatter_tiles(tab, u_tab, post):
                for g in range(NCALLS):
                    ga = gapool.tile([P, GT * CA, HID], bf, tag="ga")
                    _dma_gather_raw(
                        nc.gpsimd, nc, ga[:], tab_ap(tab, 0, LO_LIM),
                        idxA_t[:, g * wA:(g + 1) * wA], GT * capA, HID, tbw,
                        single_packet=False)
                    gb = gbpool.tile([P, GT * CB, HID], bf, tag="gb")
                    _dma_gather_raw(
                        nc.gpsimd, nc, gb[:], tab_ap(tab, HI_BASE, LO_LIM),
                        idxB_t[:, g * wB:(g + 1) * wB], GT * capB, HID, tbw,
                        single_packet=False)
                    for k in range(GT):
                        t = g * GT + k
                        sel = selpool.tile([P, NCH, P], bf, tag="sel")
                        nc.vector.tensor_tensor(
                            out=sel[:],
                            in0=dest_t[:, t * NCH:(t + 1) * NCH, None]
                                .to_broadcast([P, NCH, P]),
                            in1=iota_b[:].rearrange("p (c q) -> p c q", q=P),
                            op=mybir.AluOpType.is_equal,
                        )
                        acc = pacc.tile([P, HID], f32, space="PSUM", tag="acc")
                        for cc in range(NCH):
                            rhs = (ga[:, k * CA + cc, :] if cc < CA
                                   else gb[:, k * CB + cc - CA, :])
                            nc.tensor.matmul(out=acc[:], lhsT=sel[:, cc, :], rhs=rhs,
                                             start=(cc == 0), stop=False)
                        nc.tensor.matmul(out=acc[:], lhsT=ident[:],
                                         rhs=u_tab[:, t * HID:(t + 1) * HID],
                                         start=False, stop=True)
                        post(t, acc)

            # ---- L1 scatter + post: s2 = dinv * relu(dinv*(acc+u) + T1) ----
            def post1(t, acc):
                tmp2 = wpool.tile([P, HID], f32, tag="tmp2")
                nc.scalar.activation(out=tmp2[:], in_=acc[:],
                                     func=mybir.ActivationFunctionType.Copy,
                                     scale=dinv_t[:, t:t + 1])
                h1 = wpool.tile([P, HID], f32, tag="h1")
                nc.vector.tensor_tensor(out=h1[:], in0=tmp2[:], in1=t1_t[:],
                                        op=mybir.AluOpType.add)
                h1r = wpool.tile([P, HID], f32, tag="h1r")
                nc.scalar.activation(out=h1r[:], in_=h1[:],
                                     func=mybir.ActivationFunctionType.Relu)
                nc.scalar.activation(out=s2_t[:, t * HID:(t + 1) * HID],
                                     in_=h1r[:],
                                     func=mybir.ActivationFunctionType.Copy,
                                     scale=dinv_t[:, t:t + 1])

            scatter_tiles(s1_tab, u1_t, post1)

            nc.gpsimd.dma_start(
                out=ag2_in[:].rearrange("(t p) w -> p t w", p=P)[:, :, 0:HID],
                in_=s2_t[:].rearrange("p (t f) -> p t f", f=HID),
            )
            nc.gpsimd.collective_compute(
                "AllGather", mybir.AluOpType.bypass,
                replica_groups=[list(range(NCORES))],
                ins=[ag2_in[:]], outs=[s2_tab[:]],
            )

            # ---- L2 scatter + post ----
            def post2(t, acc):
                agg = wpool.tile([P, HID], f32, tag="agg")
                nc.scalar.activation(out=agg[:], in_=acc[:],
                                     func=mybir.ActivationFunctionType.Copy,
                                     scale=dinv_t[:, t:t + 1])
                trp = ptr.tile([HID, P], f32, space="PSUM", tag="trp")
                nc.tensor.transpose(out=trp[:], in_=agg[:], identity=ident[:])
                aggT = wpool.tile([HID, P], f32, tag="aggT")
                nc.scalar.activation(out=aggT[:], in_=trp[:],
                                     func=mybir.ActivationFunctionType.Copy)
                p3 = p3pool.tile([P, HID2], f32, space="PSUM", tag="p3")
                nc.tensor.matmul(out=p3[:], lhsT=aggT[:], rhs=w2_t[:],
                                 start=True, stop=False)
                nc.tensor.matmul(out=p3[:], lhsT=ones_row[:], rhs=t2_t[0:1, :],
                                 start=False, stop=True)
                h2 = wpool.tile([P, HID2], f32, tag="h2")
                nc.scalar.activation(out=h2[:], in_=p3[:],
                                     func=mybir.ActivationFunctionType.Relu)
                prod = wpool.tile([P, HID2], f32, tag="prod")
                nc.vector.tensor_tensor(out=prod[:], in0=h2[:], in1=fcw_t[:],
                                        op=mybir.AluOpType.mult)
                nc.vector.reduce_sum(out=out_t[:, t:t + 1], in_=prod[:],
                                     axis=mybir.AxisListType.X)

            scatter_tiles(s2_tab, s2_t, post2)

            nc.sync.dma_start(out=y_d[:], in_=out_t[:])

    nc.compile()
    return nc


# ----------------------------------------------------------------------
# entry point
# ----------------------------------------------------------------------
def prepare(inputs):
    inputs = {k: np.asarray(v) for k, v in inputs.items()}
    cores, consts = host_prep(**inputs)
    nc = build_bass(consts["CA"], consts["CB"])

    cast = BF16 if USE_BF16 else np.float32
    w2 = consts["W2p"].astype(np.float32)
    t1 = np.tile(consts["T1"][None, :], (P, 1)).astype(np.float32)
    t2 = np.tile(consts["T2"][None, :], (P, 1)).astype(np.float32)
    fcw = np.tile(consts["fcW"].reshape(1, -1), (P, 1)).astype(np.float32)

    in_maps = []
    for c in range(NCORES):
        in_maps.append({
            "xT": cores[c]["xT"].astype(BF16) if USE_BF16 else cores[c]["xT"],
            "idxA": cores[c]["idxA"],
            "idxB": cores[c]["idxB"],
            "dest": cores[c]["dest"].astype(cast),
            "dinv": cores[c]["dinv"],
            "w1": consts["W1p"].astype(BF16) if USE_BF16 else consts["W1p"],
            "w2": w2,
            "t1": t1,
            "t2": t2,
            "fcw": fcw,
        })
    return nc, in_maps, consts


def execute(nc, in_maps):
    from concourse.bass_utils import run_bass_kernel_spmd
    return run_bass_kernel_spmd(nc, in_maps, core_ids=list(range(NCORES)))


def unshard(res, consts):
    y = np.zeros((N_NODES, 1), np.float32)
    nos = consts["node_of_slot"]
    fcb = consts["fcb"]
    for c in range(NCORES):
        nodes = nos[c * SPC:(c + 1) * SPC]
        occ = nodes >= 0
        vals = res.results[c]["y"].T.reshape(-1) + fcb
        y[nodes[occ], 0] = vals[occ]
    return y


def kernel(**inputs):
    nc, in_maps, consts = prepare(inputs)
    res = execute(nc, in_maps)
    return unshard(res, consts)



# revision 12
# speedup vs baseline: 1.5026x; 1.5026x over previous
"""Distributed 2-layer GCN (BangaloreGCN) on 8 Trainium2 NeuronCores.

v2 strategy (node/graph-parallel):
  * Nodes packed into 8*49 destination bins of 128 lanes (LPT on
    in-degree).  GCN refactored so message passing is gather + one-hot
    matmul segment-sum: out = dinv*(A @ (dinv*h)) + dinv^2*h, BN scale
    folded into W, biases applied channel-major post-transpose.
  * Layer tables are DENSELY packed for the collective: L1 table is
    [NSLOT/2, 128] bf16 (two 64-wide node rows per 256B gather line),
    L2 table [NSLOT/4, 128] bf16 (four 32-wide rows per line; W2 is
    applied BEFORE the collective, as in the reference).  AllGather
    payloads are therefore 6.4MB / 3.2MB with zero padding.
  * dma_gather streams per source-class (slot mod 2 for L1, mod 4 for
    L2) with per-tile compile-time chunk budgets (max over cores).
    Source nodes are greedily class-balanced on the host to minimize
    chunk budgets.
  * Scatter: per dest tile, sel one-hot matmuls (node-major) accumulate
    into PSUM; self-loop added via identity matmul with the local table
    tile.  Post-BN runs channel-major: PE transpose then one
    Activation op (relu(zT + T_col)) per tile.
"""

import sys

sys.path.insert(0, "/opt/trn_rl_repo")

import heapq

import ml_dtypes
import numpy as np

BF16 = ml_dtypes.bfloat16

# ---- problem constants (hardcoded per contest contract) ----
N_NODES = 50000
IN_CH = 128
HID = 64
HID2 = 32
BN_EPS = 1e-5

NCORES = 8
P = 128
TILES = 49                 # dest tiles per core
SPC = TILES * P            # slots per core (6272)
NSLOT = NCORES * SPC       # 50176
NBINS = NCORES * TILES     # 392
NL1 = NSLOT // 2           # L1 gather lines (25088)
NL2 = NSLOT // 4           # L2 gather lines (12544)
GT = 7                     # dest tiles per gather group
NGRP = TILES // GT
PAD_DEST = 200.0


# ----------------------------------------------------------------------
# host-side preparation
# ----------------------------------------------------------------------
def _pack_nodes_bins(deg_in, n):
    """LPT-pack nodes into NBINS bins of <=128 by in-degree."""
    order = np.argsort(-deg_in, kind="stable")
    heap = [(0, b) for b in range(NBINS)]
    heapq.heapify(heap)
    counts = np.zeros(NBINS, np.int32)
    binof = np.empty(n, np.int32)
    for v in order:
        load, b = heapq.heappop(heap)
        binof[v] = b
        counts[b] += 1
        if counts[b] < P:
            heapq.heappush(heap, (load + int(deg_in[v]), b))
    return binof


def _assign_classes(row, col_bin, out_deg, binof, n):
    """Greedy mod-4 class per node, balancing per-(dest bin, class) edge
    counts subject to 32 lanes per class per source bin."""
    order_e = np.argsort(row, kind="stable")
    s_sorted = row[order_e]
    starts = np.searchsorted(s_sorted, np.arange(n))
    ends = np.searchsorted(s_sorted, np.arange(n) + 1)
    dbins_sorted = col_bin[order_e]
    cnt = np.zeros((NBINS, 4), np.int64)
    cap = np.full((NBINS, 4), P // 4, np.int32)
    cls = np.empty(n, np.int8)
    for v in np.argsort(-out_deg, kind="stable"):
        b = binof[v]
        db = dbins_sorted[starts[v]:ends[v]]
        if len(db):
            sc = cnt[db, :].sum(axis=0)
        else:
            sc = np.zeros(4, np.int64)
        sc = np.where(cap[b] > 0, sc, 1 << 40)
        c = int(np.argmin(sc))
        cls[v] = c
        cap[b, c] -= 1
        if len(db):
            np.add.at(cnt, (db, c), 1)
    return cls


def _wrap_idx(arr):
    ni = arr.shape[0]
    blk = arr.reshape(ni // 16, 16).T.astype(np.int16)
    return np.tile(blk, (8, 1))


def host_prep(x, edge_index, W1, b1, W2, b2, fcW, fcb,
              g1, be1, rm1, rv1, g2, be2, rm2, rv2):
    n = x.shape[0]
    row = np.asarray(edge_index[0], np.int64)
    col = np.asarray(edge_index[1], np.int64)

    deg = np.bincount(col, minlength=n).astype(np.float32) + 1.0
    dinv = (1.0 / np.sqrt(deg)).astype(np.float32)
    deg_in = np.bincount(col, minlength=n)
    deg_out = np.bincount(row, minlength=n)

    binof = _pack_nodes_bins(deg_in, n)
    cls4 = _assign_classes(row, binof[col], deg_out, binof, n)

    # lanes: class c gets lanes {c, c+4, ...} within its bin
    lane = np.empty(n, np.int64)
    key = binof.astype(np.int64) * 4 + cls4
    order = np.argsort(key, kind="stable")
    uniq, first = np.unique(key[order], return_index=True)
    # rank within (bin, class)
    rank = np.arange(n) - np.repeat(first, np.diff(np.append(first, n)))
    lane[order] = cls4[order] + 4 * rank
    assert lane.max() < P

    # per-core: sort own bins desc by in-degree -> tile slots; the table /
    # collective layout is core-major tile-slot order.
    bin_in = np.bincount(binof[col], minlength=NBINS)
    tslot_of_bin = np.empty(NBINS, np.int64)
    for c in range(NCORES):
        bins = np.arange(c * TILES, (c + 1) * TILES)
        order_b = bins[np.argsort(-bin_in[bins], kind="stable")]
        tslot_of_bin[order_b] = np.arange(TILES)
    rbin = (np.arange(NBINS) // TILES) * TILES + tslot_of_bin

    slot_of_node = rbin[binof] * P + lane          # table slot
    node_of_slot = np.full(NSLOT, -1, np.int64)
    node_of_slot[slot_of_node] = np.arange(n)

    src_slot = slot_of_node[row]
    dst_slot = slot_of_node[col]
    dbin = dst_slot // P
    dlane = dst_slot % P
    scls = (src_slot % 4).astype(np.int64)

    core_of_edge = dbin // TILES
    tslot = dbin % TILES

    # per (core, tslot, cls4) counts -> budgets (max over cores)
    cnt4 = np.zeros((NCORES, TILES, 4), np.int64)
    np.add.at(cnt4, (core_of_edge, tslot, scls), 1)
    c2_budget = -(-cnt4.max(axis=0) // P)                 # [TILES, 4] chunks
    cnt2 = cnt4[:, :, 0::2].sum(-1), cnt4[:, :, 1::2].sum(-1)
    c1_budget = np.stack(
        [-(-cnt2[0].max(axis=0) // P), -(-cnt2[1].max(axis=0) // P)], 1)  # [TILES, 2]

    # sort edges by (core, tslot, class) once; slice per group
    ekey = ((core_of_edge * TILES + tslot) * 4 + scls)
    eorder = np.argsort(ekey, kind="stable")
    e_src = src_slot[eorder]
    e_dlane = dlane[eorder]
    e_key = ekey[eorder]
    bounds = np.searchsorted(e_key, np.arange(NCORES * TILES * 4 + 1))

    S1c = (g1 / np.sqrt(rv1 + BN_EPS)).astype(np.float32)
    T1 = ((b1 - rm1) * S1c + be1).astype(np.float32)
    S2c = (g2 / np.sqrt(rv2 + BN_EPS)).astype(np.float32)
    T2 = ((b2 - rm2) * S2c + be2).astype(np.float32)
    W1p = (W1 * S1c[None, :]).astype(np.float32)
    W2p = (W2 * S2c[None, :]).astype(np.float32)

    NCH1 = int(c1_budget.sum())       # total L1 chunk-columns across tiles
    NCH2 = int(c2_budget.sum())

    cores = []
    for c in range(NCORES):
        idx1 = [np.zeros(int(c1_budget[:, a].sum()) * P, np.int64) for a in range(2)]
        idx2 = [np.zeros(int(c2_budget[:, k].sum()) * P, np.int64) for k in range(4)]
        dest1 = np.full((NCH1, P), PAD_DEST, np.float32)
        dest2 = np.full((NCH2, P), PAD_DEST, np.float32)
        off1 = [0, 0]
        off2 = [0, 0, 0, 0]
        col1 = 0
        col2 = 0
        for t in range(TILES):
            for k in range(4):
                lo, hi = bounds[(c * TILES + t) * 4 + k], bounds[(c * TILES + t) * 4 + k + 1]
                src_k = e_src[lo:hi]
                dl_k = e_dlane[lo:hi]
                cap = int(c2_budget[t, k]) * P
                assert hi - lo <= cap
                idx2[k][off2[k]:off2[k] + (hi - lo)] = src_k >> 2
                d = dest2[col2:col2 + c2_budget[t, k]].reshape(-1)
                d[:hi - lo] = dl_k
                off2[k] += cap
                col2 += int(c2_budget[t, k])
            for a in range(2):
                parts = []
                for k in (a, a + 2):
                    lo, hi = bounds[(c * TILES + t) * 4 + k], bounds[(c * TILES + t) * 4 + k + 1]
                    parts.append((e_src[lo:hi], e_dlane[lo:hi]))
                src_a = np.concatenate([p[0] for p in parts])
                dl_a = np.concatenate([p[1] for p in parts])
                cap = int(c1_budget[t, a]) * P
                assert len(src_a) <= cap
                idx1[a][off1[a]:off1[a] + len(src_a)] = src_a >> 1
                d = dest1[col1:col1 + c1_budget[t, a]].reshape(-1)
                d[:len(src_a)] = dl_a
                off1[a] += cap
                col1 += int(c1_budget[t, a])

        nodes = node_of_slot[c * SPC:(c + 1) * SPC]
        occ = nodes >= 0
        xs = np.zeros((SPC, IN_CH), np.float32)
        xs[occ] = x[nodes[occ]]
        dv = np.zeros(SPC, np.float32)
        dv[occ] = dinv[nodes[occ]]

        cores.append(dict(
            idx1=[_wrap_idx(v) for v in idx1],
            idx2=[_wrap_idx(v) for v in idx2],
            dest1=dest1.T.astype(BF16).copy(),   # [P, NCH1]
            dest2=dest2.T.astype(BF16).copy(),
            xT=np.ascontiguousarray(xs.T),
            dinv=np.ascontiguousarray(dv.reshape(TILES, P).T),
            nodes=nodes,
        ))

    consts = dict(W1p=W1p, W2p=W2p, T1=T1, T2=T2,
                  fcW=np.asarray(fcW, np.float32),
                  fcb=float(np.asarray(fcb).reshape(-1)[0]),
                  c1=c1_budget, c2=c2_budget)
    return cores, consts


# ----------------------------------------------------------------------
# device program
# ----------------------------------------------------------------------
def _dma_gather_raw(gp, bassmod, out_ap, in_ap, idxs_ap, num_idxs, elem_size,
                    elem_step, single_packet=True, queue_num=0):
    """bass.dma_gather with elem_size_bytes below 256B allowed (stride must
    still be a multiple of 256B)."""
    import concourse.mybir as mybir
    from concourse import ap_utils
    from concourse.bass import MemorySpace, exact_div, round_up_to_multiple

    assert idxs_ap.dtype == mybir.dt.int16
    assert in_ap.dtype == out_ap.dtype
    assert in_ap.space == MemorySpace.DRAM
    assert idxs_ap.space == MemorySpace.SBUF and out_ap.space == MemorySpace.SBUF
    assert ap_utils.ap_is_contiguous(out_ap.ap[1:])
    assert ap_utils.ap_is_contiguous(idxs_ap.ap[1:])
    assert in_ap.ap[-1][1] == out_ap.ap[-1][1] == elem_size
    assert out_ap.ap[0][1] * out_ap.ap[1][1] == round_up_to_multiple(num_idxs, 128)
    assert in_ap.ap[0][0] == elem_step
    stride_bytes_256 = exact_div(elem_step * mybir.dt.size(in_ap.dtype), 256)
    assert stride_bytes_256 < 256
    return gp.add_instruction(
        mybir.InstDMAGatherAnt(
            name=bassmod.get_next_instruction_name(),
            ins=[*gp.lower_ap_dma(in_ap, for_custom_bir_dma=True),
                 gp.lower_ap(idxs_ap),
                 gp.lower_val_access(gp.to_reg(num_idxs))],
            outs=[gp.lower_ap(out_ap)],
            transpose=False,
            num_idxs=num_idxs,
            elem_size=elem_size,
            stride_bytes_256=stride_bytes_256,
            gen_mode=0,
            single_packet=single_packet,
            queue_num=queue_num,
            sbuf_tokens_per_rank=0,
            sbuf_free_dim_per_rank=0,
            sbuf_free_dim_pad_per_rank=0,
            sbuf_byte_offset=0,
        ))


def build_bass(c1, c2):
    import concourse.bacc as bacc
    import concourse.bass as bassm
    import concourse.mybir as mybir
    import concourse.tile as tile
    from concourse.library_config import mlp
    from concourse.masks import make_identity

    f32 = mybir.dt.float32
    bf = mybir.dt.bfloat16
    i16 = mybir.dt.int16
    AF = mybir.ActivationFunctionType

    c1 = [list(map(int, r)) for r in c1]      # [TILES][2]
    c2 = [list(map(int, r)) for r in c2]      # [TILES][4]
    NCH1 = [sum(r) for r in c1]
    NCH2 = [sum(r) for r in c2]
    NCHM = max(max(NCH1), max(NCH2))
    w1tot = [sum(c1[t][a] for t in range(TILES)) * 8 for a in range(2)]
    w2tot = [sum(c2[t][k] for t in range(TILES)) * 8 for k in range(4)]

    nc = bacc.Bacc("TRN2", target_bir_lowering=False)
    xT_d = nc.dram_tensor("xT", [P, SPC], bf, kind="ExternalInput")
    idx1_d = [nc.dram_tensor(f"idx1_{a}", [P, w1tot[a]], i16, kind="ExternalInput")
              for a in range(2)]
    idx2_d = [nc.dram_tensor(f"idx2_{k}", [P, w2tot[k]], i16, kind="ExternalInput")
              for k in range(4)]
    dest1_d = nc.dram_tensor("dest1", [P, sum(NCH1)], bf, kind="ExternalInput")
    dest2_d = nc.dram_tensor("dest2", [P, sum(NCH2)], bf, kind="ExternalInput")
    dinv_d = nc.dram_tensor("dinv", [P, TILES], f32, kind="ExternalInput")
    w1_d = nc.dram_tensor("w1", [IN_CH, HID], bf, kind="ExternalInput")
    w2_d = nc.dram_tensor("w2", [HID, HID2], f32, kind="ExternalInput")
    t1_d = nc.dram_tensor("t1", [HID, 1], f32, kind="ExternalInput")
    t2_d = nc.dram_tensor("t2", [HID2, 1], f32, kind="ExternalInput")
    fcw_d = nc.dram_tensor("fcw", [HID2, 1], f32, kind="ExternalInput")
    y_d = nc.dram_tensor("y", [P, TILES], f32, kind="ExternalOutput")

    with tile.TileContext(nc) as tc:
        with (
            tc.tile_pool(name="const", bufs=1) as cpool,
            tc.tile_pool(name="upart", bufs=1) as upool,
            tc.tile_pool(name="g1", bufs=2) as g1pool,
            tc.tile_pool(name="g2", bufs=2) as g2pool,
            tc.tile_pool(name="sel", bufs=8) as selpool,
            tc.tile_pool(name="work", bufs=6) as wpool,
            tc.tile_pool(name="pacc", bufs=3, space="PSUM") as pacc,
            tc.tile_pool(name="pmm", bufs=2, space="PSUM") as pmm,
            tc.tile_pool(name="ptr", bufs=2, space="PSUM") as ptr,
            tc.tile_pool(name="dram", bufs=1, space="DRAM") as dpool,
        ):
            nc.gpsimd.load_library(mlp)

            # ---- constants ----
            idx1_t = [cpool.tile([P, w1tot[a]], i16, name=f"idx1t{a}")
                      for a in range(2)]
            idx2_t = [cpool.tile([P, w2tot[k]], i16, name=f"idx2t{k}")
                      for k in range(4)]
            for a in range(2):
                nc.sync.dma_start(out=idx1_t[a][:], in_=idx1_d[a][:])
            for k in range(4):
                nc.sync.dma_start(out=idx2_t[k][:], in_=idx2_d[k][:])
            dest1_t = cpool.tile([P, sum(NCH1)], bf)
            nc.sync.dma_start(out=dest1_t[:], in_=dest1_d[:])
            dest2_t = cpool.tile([P, sum(NCH2)], bf)
            nc.sync.dma_start(out=dest2_t[:], in_=dest2_d[:])
            dinv_t = cpool.tile([P, TILES], f32)
            nc.sync.dma_start(out=dinv_t[:], in_=dinv_d[:])
            w1_t = cpool.tile([IN_CH, HID], bf)
            nc.sync.dma_start(out=w1_t[:], in_=w1_d[:])
            w2_t = cpool.tile([HID, HID2], f32)
            nc.sync.dma_start(out=w2_t[:], in_=w2_d[:])
            t1_t = cpool.tile([HID, 1], f32)
            nc.sync.dma_start(out=t1_t[:], in_=t1_d[:])
            t2_t = cpool.tile([HID2, 1], f32)
            nc.sync.dma_start(out=t2_t[:], in_=t2_d[:])
            fcw_t = cpool.tile([HID2, 1], f32)
            nc.sync.dma_start(out=fcw_t[:], in_=fcw_d[:])

            identf = cpool.tile([P, P], f32)
            make_identity(nc, identf[:])
            identb = cpool.tile([P, P], bf)
            nc.vector.tensor_copy(out=identb[:], in_=identf[:])
            # iota2[p, j, c] = j  (lane index on middle axis, chunk innermost)
            iota_i = cpool.tile([P, P * NCHM], mybir.dt.int32)
            nc.gpsimd.iota(iota_i[:], pattern=[[1, P], [0, NCHM]], base=0,
                           channel_multiplier=0)
            iota_b = cpool.tile([P, P * NCHM], bf)
            nc.vector.tensor_copy(out=iota_b[:], in_=iota_i[:])

            tab1_t = upool.tile([P, TILES, HID], bf, tag="tab1")
            tab2_t = upool.tile([P, TILES, HID2], bf, tag="tab2")
            out_t = upool.tile([P, TILES], f32, tag="out")

            ag1_in = dpool.tile([SPC, HID], bf)
            s1_tab = dpool.tile([NL1, 2 * HID], bf, addr_space="Shared")
            ag2_in = dpool.tile([SPC, HID2], bf)
            s2_tab = dpool.tile([NL2, 4 * HID2], bf, addr_space="Shared")

            # ---- L1 dense: tab1 = dinv * ((x*dinv) @ W1') ----
            xfull = cpool.tile([P, SPC], bf)
            nc.sync.dma_start(out=xfull[:], in_=xT_d[:])
            for t in range(TILES):
                pm = pmm.tile([P, HID], f32, space="PSUM", tag="pm")
                nc.tensor.matmul(out=pm[:], lhsT=xfull[:, t * P:(t + 1) * P],
                                 rhs=w1_t[:], start=True, stop=True)
                nc.scalar.activation(out=tab1_t[:, t, :], in_=pm[:],
                                     func=AF.Copy, scale=dinv_t[:, t:t + 1])

            nc.sync.dma_start(
                out=ag1_in[:].rearrange("(t p) w -> p t w", p=P),
                in_=tab1_t[:],
            )
            nc.gpsimd.collective_compute(
                "AllGather", mybir.AluOpType.bypass,
                replica_groups=[list(range(NCORES))],
                ins=[ag1_in[:]], outs=[s1_tab[:]],
            )

            def tab_ap(tab, nlines, sub_off, elem):
                return bassm.AP(tensor=tab[:].tensor, offset=sub_off,
                                ap=[[2 * HID, nlines], [1, elem]])

            # ---- generic scatter over one layer ----
            def scatter(tab, nlines, elem, ncls, cbud, idx_t, dest_t, ga_pool,
                        nch, self_rhs, post):
                iota_r = iota_b[:].rearrange("p (j c) -> p j c", c=NCHM)
                goff = [0] * ncls          # running idx-column offset per class
                dcol = 0                   # running dest-img column
                gmax = [max(sum(cbud[g * GT + j][k] for j in range(GT))
                            for g in range(NGRP)) for k in range(ncls)]
                for g in range(NGRP):
                    gw = [sum(cbud[g * GT + j][k] for j in range(GT))
                          for k in range(ncls)]
                    gas = []
                    for k in range(ncls):
                        ga = ga_pool.tile([P, gmax[k], elem], bf, tag=f"ga{k}")
                        ni = gw[k] * P
                        _dma_gather_raw(
                            nc.gpsimd, nc,
                            ga[:, :gw[k], :],
                            tab_ap(tab, nlines, k * elem, elem),
                            idx_t[k][:, goff[k]:goff[k] + ni // 16],
                            ni, elem, 2 * HID, single_packet=False)
                        goff[k] += ni // 16
                        gas.append(ga)
                    coff = [0] * ncls
                    for j in range(GT):
                        t = g * GT + j
                        nch_t = sum(cbud[t])
                        sel = selpool.tile([P, P, NCHM], bf, tag="sel")
                        nc.vector.tensor_tensor(
                            out=sel[:, :, :nch_t],
                            in0=dest_t[:, None, dcol:dcol + nch_t]
                                .to_broadcast([P, P, nch_t]),
                            in1=iota_r[:, :, :nch_t],
                            op=mybir.AluOpType.is_equal,
                        )
                        acc = pacc.tile([P, elem], f32, space="PSUM", tag="acc")
                        cc = 0
                        for k in range(ncls):
                            for i in range(cbud[t][k]):
                                nc.tensor.matmul(
                                    out=acc[:], lhsT=sel[:, :, cc],
                                    rhs=gas[k][:, coff[k] + i, :],
                                    start=(cc == 0), stop=False)
                                cc += 1
                            coff[k] += cbud[t][k]
                        nc.tensor.matmul(out=acc[:], lhsT=identb[:],
                                         rhs=self_rhs(t), start=False, stop=True)
                        dcol += nch_t
                        post(t, acc)

            # ---- L1 scatter + post ----
            def post1(t, acc):
                z = wpool.tile([P, HID], f32, tag="z1")
                nc.scalar.activation(out=z[:], in_=acc[:], func=AF.Copy,
                                     scale=dinv_t[:, t:t + 1])
                zt = ptr.tile([HID, P], f32, space="PSUM", tag="zt")
                nc.tensor.transpose(out=zt[:], in_=z[:], identity=identf[:])
                h2 = wpool.tile([HID, P], f32, tag="h2T")
                nc.scalar.activation(out=h2[:], in_=zt[:], func=AF.Relu,
                                     bias=t1_t[:])
                p2 = pmm.tile([P, HID2], f32, space="PSUM", tag="pm")
                nc.tensor.matmul(out=p2[:], lhsT=h2[:], rhs=w2_t[:],
                                 start=True, stop=True)
                nc.scalar.activation(out=tab2_t[:, t, :], in_=p2[:], func=AF.Copy,
                                     scale=dinv_t[:, t:t + 1])

            scatter(s1_tab, NL1, HID, 2, c1, idx1_t, dest1_t, g1pool,
                    NCH1, lambda t: tab1_t[:, t, :], post1)

            nc.sync.dma_start(
                out=ag2_in[:].rearrange("(t p) w -> p t w", p=P),
                in_=tab2_t[:],
            )
            nc.gpsimd.collective_compute(
                "AllGather", mybir.AluOpType.bypass,
                replica_groups=[list(range(NCORES))],
                ins=[ag2_in[:]], outs=[s2_tab[:]],
            )

            # ---- L2 scatter + post ----
            def post2(t, acc):
                z = wpool.tile([P, HID2], f32, tag="z2")
                nc.scalar.activation(out=z[:], in_=acc[:], func=AF.Copy,
                                     scale=dinv_t[:, t:t + 1])
                zt = ptr.tile([HID2, P], f32, space="PSUM", tag="zt")
                nc.tensor.transpose(out=zt[:], in_=z[:], identity=identf[:])
                h3 = wpool.tile([HID2, P], f32, tag="h3T")
                nc.scalar.activation(out=h3[:], in_=zt[:], func=AF.Relu,
                                     bias=t2_t[:])
                py = pmm.tile([P, 1], f32, space="PSUM", tag="pm")
                nc.tensor.matmul(out=py[:], lhsT=h3[:], rhs=fcw_t[:],
                                 start=True, stop=True)
                nc.scalar.activation(out=out_t[:, t:t + 1], in_=py[:],
                                     func=AF.Copy)

            scatter(s2_tab, NL2, HID2, 4, c2, idx2_t, dest2_t, g2pool,
                    NCH2, lambda t: tab2_t[:, t, :], post2)

            nc.sync.dma_start(out=y_d[:], in_=out_t[:])

    nc.compile()
    return nc


# ----------------------------------------------------------------------
# entry point
# ----------------------------------------------------------------------
def prepare(inputs):
    inputs = {k: np.asarray(v) for k, v in inputs.items()}
    cores, consts = host_prep(**inputs)
    nc = build_bass(consts["c1"], consts["c2"])

    t1 = consts["T1"].reshape(HID, 1).astype(np.float32)
    t2 = consts["T2"].reshape(HID2, 1).astype(np.float32)
    fcw = consts["fcW"].reshape(HID2, 1).astype(np.float32)

    in_maps = []
    for c in range(NCORES):
        m = {
            "xT": cores[c]["xT"].astype(BF16),
            "dest1": cores[c]["dest1"],
            "dest2": cores[c]["dest2"],
            "dinv": cores[c]["dinv"],
            "w1": consts["W1p"].astype(BF16),
            "w2": consts["W2p"].astype(np.float32),
            "t1": t1,
            "t2": t2,
            "fcw": fcw,
        }
        for a in range(2):
            m[f"idx1_{a}"] = cores[c]["idx1"][a]
        for k in range(4):
            m[f"idx2_{k}"] = cores[c]["idx2"][k]
        in_maps.append(m)
    return nc, in_maps, consts | {"cores": cores}


def execute(nc, in_maps):
    from concourse.bass_utils import run_bass_kernel_spmd
    return run_bass_kernel_spmd(nc, in_maps, core_ids=list(range(NCORES)))


def unshard(res, consts):
    y = np.zeros((N_NODES, 1), np.float32)
    fcb = consts["fcb"]
    for c in range(NCORES):
        nodes = consts["cores"][c]["nodes"]
        occ = nodes >= 0
        vals = res.results[c]["y"].T.reshape(-1) + fcb
        y[nodes[occ], 0] = vals[occ]
    return y


def kernel(**inputs):
    nc, in_maps, consts = prepare(inputs)
    res = execute(nc, in_maps)
    return unshard(res, consts)


# revision 25
# speedup vs baseline: 1.6608x; 1.1053x over previous
"""Distributed 2-layer GCN (BangaloreGCN) on 8 Trainium2 NeuronCores.

v3 strategy (node/graph-parallel):
  * Nodes packed into 8*49 destination bins of 128 lanes (LPT on
    in-degree).  GCN refactored so message passing is gather + one-hot
    matmul segment-sum: out = dinv*(A @ (dinv*h)) + dinv^2*h, BN scale
    folded into W, biases applied channel-major after a PE transpose.
  * Layer tables are densely packed (2 or 4 node rows per 256B gather
    line) and replicated with AllGather collectives split into two
    source-halves (tile slots 0-23 / 24-48) so the second collective
    overlaps compute:
      - L1: AG1A starts right after the first 24 dense tiles; the L1
        scatter is two passes (A-half sources while AG1B transfers,
        then B-half sources + PSUM-spill re-add).
      - L2: W2 applied before the collective (32-wide table); AG2A is
        issued mid-way through the L1 B-pass, one scatter pass.
  * dma_gather streams per source-class (slot parity) with per-tile
    compile-time chunk budgets (max over cores); greedy host-side
    class balancing.  Sel one-hot masks are built chunk-innermost so
    the DVE 2x perf mode applies.
"""

import sys

sys.path.insert(0, "/opt/trn_rl_repo")

import heapq

import ml_dtypes
import numpy as np

BF16 = ml_dtypes.bfloat16

# ---- problem constants (hardcoded per contest contract) ----
N_NODES = 50000
IN_CH = 128
HID = 64
HID2 = 32
BN_EPS = 1e-5

NCORES = 8
P = 128
TILES = 49                 # dest tiles per core
SPC = TILES * P            # slots per core (6272)
NSLOT = NCORES * SPC       # 50176
NBINS = NCORES * TILES     # 392
TA = 24                    # tiles in half A
TB = TILES - TA            # 25
SZA = TA * P               # 3072 slots per core in half A
SZB = TB * P               # 3200
NLA1 = NCORES * SZA // 2   # 12288 L1 lines in table A
NLB1 = NCORES * SZB // 2   # 12800
NL2 = NSLOT // 4           # 12544 L2 lines (A region then B region)
NL2A = NCORES * SZA // 4   # 6144
PAD_DEST = 200.0


# ----------------------------------------------------------------------
# host-side preparation
# ----------------------------------------------------------------------
def _pack_nodes_bins(deg_in, n):
    """LPT-pack nodes into NBINS bins of <=128 by in-degree."""
    order = np.argsort(-deg_in, kind="stable")
    heap = [(0, b) for b in range(NBINS)]
    heapq.heapify(heap)
    counts = np.zeros(NBINS, np.int32)
    binof = np.empty(n, np.int32)
    for v in order:
        load, b = heapq.heappop(heap)
        binof[v] = b
        counts[b] += 1
        if counts[b] < P:
            heapq.heappush(heap, (load + int(deg_in[v]), b))
    return binof


def _assign_classes(row, col_bin, out_deg, binof, h_node, n):
    """Greedy mod-4 class per node, balancing per-(dest bin, class,
    source-half) edge counts subject to 32 lanes per class per bin."""
    order_e = np.argsort(row, kind="stable")
    s_sorted = row[order_e]
    starts = np.searchsorted(s_sorted, np.arange(n))
    ends = np.searchsorted(s_sorted, np.arange(n) + 1)
    dbins_sorted = col_bin[order_e]
    cnt = np.zeros((NBINS, 4, 2), np.int64)
    cap = np.full((NBINS, 4), P // 4, np.int32)
    cls = np.empty(n, np.int8)
    for v in np.argsort(-out_deg, kind="stable"):
        b = binof[v]
        h = h_node[v]
        db = dbins_sorted[starts[v]:ends[v]]
        if len(db):
            sc = cnt[db, :, h].sum(axis=0)
        else:
            sc = np.zeros(4, np.int64)
        sc = np.where(cap[b] > 0, sc, 1 << 40)
        c = int(np.argmin(sc))
        cls[v] = c
        cap[b, c] -= 1
        if len(db):
            np.add.at(cnt, (db, c, h), 1)
    return cls


def _wrap_idx(arr):
    ni = arr.shape[0]
    blk = arr.reshape(ni // 16, 16).T.astype(np.int16)
    return np.tile(blk, (8, 1))


def host_prep(x, edge_index, W1, b1, W2, b2, fcW, fcb,
              g1, be1, rm1, rv1, g2, be2, rm2, rv2):
    n = x.shape[0]
    row = np.asarray(edge_index[0], np.int64)
    col = np.asarray(edge_index[1], np.int64)

    deg = np.bincount(col, minlength=n).astype(np.float32) + 1.0
    dinv = (1.0 / np.sqrt(deg)).astype(np.float32)
    deg_in = np.bincount(col, minlength=n)
    deg_out = np.bincount(row, minlength=n)

    binof = _pack_nodes_bins(deg_in, n)

    # per-core: sort own bins desc by in-degree -> tile slots
    bin_in = np.bincount(binof[col], minlength=NBINS)
    tslot_of_bin = np.empty(NBINS, np.int64)
    for c in range(NCORES):
        bins = np.arange(c * TILES, (c + 1) * TILES)
        order_b = bins[np.argsort(-bin_in[bins], kind="stable")]
        tslot_of_bin[order_b] = np.arange(TILES)
    rbin = (np.arange(NBINS) // TILES) * TILES + tslot_of_bin

    h_node = (tslot_of_bin[binof] >= TA).astype(np.int8)   # source half per node
    cls4 = _assign_classes(row, rbin[binof[col]], deg_out, binof, h_node, n)

    # lanes: class c gets lanes {c, c+4, ...} within its bin
    lane = np.empty(n, np.int64)
    key = binof.astype(np.int64) * 4 + cls4
    order = np.argsort(key, kind="stable")
    uniq, first = np.unique(key[order], return_index=True)
    rank = np.arange(n) - np.repeat(first, np.diff(np.append(first, n)))
    lane[order] = cls4[order] + 4 * rank
    assert lane.max() < P

    slot_of_node = rbin[binof] * P + lane          # table slot (core-major)
    node_of_slot = np.full(NSLOT, -1, np.int64)
    node_of_slot[slot_of_node] = np.arange(n)

    src_slot = slot_of_node[row]
    dst_slot = slot_of_node[col]
    dlane = dst_slot % P
    scls = (src_slot % 4).astype(np.int64)

    core_of_edge = (dst_slot // P) // TILES
    tslot = (dst_slot // P) % TILES

    # source half + half-local slot / lines
    s_core = src_slot // SPC
    s_t = (src_slot % SPC) // P
    s_lane = src_slot % P
    s_h = (s_t >= TA).astype(np.int64)
    sH = np.where(s_h == 0,
                  s_core * SZA + s_t * P + s_lane,
                  s_core * SZB + (s_t - TA) * P + s_lane)
    line1 = sH >> 1                                # within half table
    line2 = (sH >> 2) + s_h * NL2A                 # global L2 line

    # budgets
    cnt1 = np.zeros((NCORES, TILES, 2, 2), np.int64)   # [core, t, cls2, half]
    np.add.at(cnt1, (core_of_edge, tslot, scls % 2, s_h), 1)
    c1h = -(-cnt1.max(axis=0) // P)                    # [TILES, 2, 2]
    cnt4 = np.zeros((NCORES, TILES, 4), np.int64)
    np.add.at(cnt4, (core_of_edge, tslot, scls), 1)
    c2_budget = -(-cnt4.max(axis=0) // P)              # [TILES, 4]

    # sort edges by (core, tslot, cls4, half)
    ekey = (((core_of_edge * TILES + tslot) * 4 + scls) * 2 + s_h)
    eorder = np.argsort(ekey, kind="stable")
    e_line1 = line1[eorder]
    e_line2 = line2[eorder]
    e_dlane = dlane[eorder]
    e_key = ekey[eorder]
    bounds = np.searchsorted(e_key, np.arange(NCORES * TILES * 8 + 1))

    S1c = (g1 / np.sqrt(rv1 + BN_EPS)).astype(np.float32)
    T1 = ((b1 - rm1) * S1c + be1).astype(np.float32)
    S2c = (g2 / np.sqrt(rv2 + BN_EPS)).astype(np.float32)
    T2 = ((b2 - rm2) * S2c + be2).astype(np.float32)
    W1p = (W1 * S1c[None, :]).astype(np.float32)
    W2p = (W2 * S2c[None, :]).astype(np.float32)

    NCH1 = [int(c1h[:, :, h].sum()) for h in range(2)]   # dest-img cols per half
    NCH2 = int(c2_budget.sum())

    cores = []
    for c in range(NCORES):
        idx1 = [[np.zeros(int(c1h[:, a, h].sum()) * P, np.int64)
                 for h in range(2)] for a in range(2)]
        idx2 = [np.zeros(int(c2_budget[:, k].sum()) * P, np.int64)
                for k in range(4)]
        dest1 = [np.full((NCH1[h], P), PAD_DEST, np.float32) for h in range(2)]
        dest2 = np.full((NCH2, P), PAD_DEST, np.float32)
        off1 = [[0, 0], [0, 0]]
        off2 = [0, 0, 0, 0]
        col1 = [0, 0]
        col2 = 0

        def sl(t, k, h):
            i = ((c * TILES + t) * 4 + k) * 2 + h
            return bounds[i], bounds[i + 1]

        for t in range(TILES):
            for k in range(4):
                lo0, hi0 = sl(t, k, 0)
                lo1, hi1 = sl(t, k, 1)
                li = np.concatenate([e_line2[lo0:hi0], e_line2[lo1:hi1]])
                dl = np.concatenate([e_dlane[lo0:hi0], e_dlane[lo1:hi1]])
                cap = int(c2_budget[t, k]) * P
                assert len(li) <= cap
                idx2[k][off2[k]:off2[k] + len(li)] = li
                d = dest2[col2:col2 + c2_budget[t, k]].reshape(-1)
                d[:len(li)] = dl
                off2[k] += cap
                col2 += int(c2_budget[t, k])
            for h in range(2):
                for a in range(2):
                    parts = []
                    for k in (a, a + 2):
                        lo, hi = sl(t, k, h)
                        parts.append((e_line1[lo:hi], e_dlane[lo:hi]))
                    li = np.concatenate([p[0] for p in parts])
                    dl = np.concatenate([p[1] for p in parts])
                    cap = int(c1h[t, a, h]) * P
                    assert len(li) <= cap
                    idx1[a][h][off1[a][h]:off1[a][h] + len(li)] = li
                    d = dest1[h][col1[h]:col1[h] + c1h[t, a, h]].reshape(-1)
                    d[:len(li)] = dl
                    off1[a][h] += cap
                    col1[h] += int(c1h[t, a, h])

        nodes = node_of_slot[c * SPC:(c + 1) * SPC]
        occ = nodes >= 0
        xs = np.zeros((SPC, IN_CH), np.float32)
        xs[occ] = x[nodes[occ]]
        dv = np.zeros(SPC, np.float32)
        dv[occ] = dinv[nodes[occ]]

        cores.append(dict(
            idx1=[[_wrap_idx(idx1[a][h]) for h in range(2)] for a in range(2)],
            idx2=[_wrap_idx(v) for v in idx2],
            dest1=[dest1[h].T.astype(BF16).copy() for h in range(2)],
            dest2=dest2.T.astype(BF16).copy(),
            xT=np.ascontiguousarray(xs.T),
            dinv=np.ascontiguousarray(dv.reshape(TILES, P).T),
            nodes=nodes,
        ))

    consts = dict(W1p=W1p, W2p=W2p, T1=T1, T2=T2,
                  fcW=np.asarray(fcW, np.float32),
                  fcb=float(np.asarray(fcb).reshape(-1)[0]),
                  c1h=c1h, c2=c2_budget)
    return cores, consts


# ----------------------------------------------------------------------
# device program
# ----------------------------------------------------------------------
def _dma_gather_raw(gp, bassmod, out_ap, in_ap, idxs_ap, num_idxs, elem_size,
                    elem_step, single_packet=True, queue_num=0):
    """bass.dma_gather with elem_size_bytes below 256B allowed (stride must
    still be a multiple of 256B)."""
    import concourse.mybir as mybir
    from concourse import ap_utils
    from concourse.bass import MemorySpace, exact_div, round_up_to_multiple

    assert idxs_ap.dtype == mybir.dt.int16
    assert in_ap.dtype == out_ap.dtype
    assert in_ap.space == MemorySpace.DRAM
    assert idxs_ap.space == MemorySpace.SBUF and out_ap.space == MemorySpace.SBUF
    assert ap_utils.ap_is_contiguous(out_ap.ap[1:])
    assert ap_utils.ap_is_contiguous(idxs_ap.ap[1:])
    assert in_ap.ap[-1][1] == out_ap.ap[-1][1] == elem_size
    assert out_ap.ap[0][1] * out_ap.ap[1][1] == round_up_to_multiple(num_idxs, 128)
    assert in_ap.ap[0][0] == elem_step
    stride_bytes_256 = exact_div(elem_step * mybir.dt.size(in_ap.dtype), 256)
    assert stride_bytes_256 < 256
    return gp.add_instruction(
        mybir.InstDMAGatherAnt(
            name=bassmod.get_next_instruction_name(),
            ins=[*gp.lower_ap_dma(in_ap, for_custom_bir_dma=True),
                 gp.lower_ap(idxs_ap),
                 gp.lower_val_access(gp.to_reg(num_idxs))],
            outs=[gp.lower_ap(out_ap)],
            transpose=False,
            num_idxs=num_idxs,
            elem_size=elem_size,
            stride_bytes_256=stride_bytes_256,
            gen_mode=0,
            single_packet=single_packet,
            queue_num=queue_num,
            sbuf_tokens_per_rank=0,
            sbuf_free_dim_per_rank=0,
            sbuf_free_dim_pad_per_rank=0,
            sbuf_byte_offset=0,
        ))


GRP1 = [10, 10, 10, 10, 9]     # gather groups (tiles per group)
GRP1B = GRP1


def build_bass(c1h, c2):
    import concourse.bacc as bacc
    import concourse.bass as bassm
    import concourse.mybir as mybir
    import concourse.tile as tile
    from concourse.library_config import mlp
    from concourse.masks import make_identity

    f32 = mybir.dt.float32
    bf = mybir.dt.bfloat16
    i16 = mybir.dt.int16
    AF = mybir.ActivationFunctionType

    c1h = [[[int(c1h[t][a][h]) for h in range(2)] for a in range(2)]
           for t in range(TILES)]
    c2 = [list(map(int, r)) for r in c2]
    nch1 = [[sum(c1h[t][a][h] for a in range(2)) for h in range(2)]
            for t in range(TILES)]
    nch2 = [sum(r) for r in c2]
    NCH1 = [sum(nch1[t][h] for t in range(TILES)) for h in range(2)]
    NCH2 = sum(nch2)
    NCHM = max(max(max(nch1[t]) for t in range(TILES)), max(nch2))
    w1tot = [[sum(c1h[t][a][h] for t in range(TILES)) * 8 for h in range(2)]
             for a in range(2)]
    w2tot = [sum(c2[t][k] for t in range(TILES)) * 8 for k in range(4)]

    nc = bacc.Bacc("TRN2", target_bir_lowering=False)
    xT_d = nc.dram_tensor("xT", [P, SPC], bf, kind="ExternalInput")
    idx1_d = [[nc.dram_tensor(f"idx1_{a}_{h}", [P, w1tot[a][h]], i16,
                              kind="ExternalInput") for h in range(2)]
              for a in range(2)]
    idx2_d = [nc.dram_tensor(f"idx2_{k}", [P, w2tot[k]], i16,
                             kind="ExternalInput") for k in range(4)]
    dest1_d = [nc.dram_tensor(f"dest1_{h}", [P, NCH1[h]], bf,
                              kind="ExternalInput") for h in range(2)]
    dest2_d = nc.dram_tensor("dest2", [P, NCH2], bf, kind="ExternalInput")
    dinv_d = nc.dram_tensor("dinv", [P, TILES], f32, kind="ExternalInput")
    w1_d = nc.dram_tensor("w1", [IN_CH, HID], bf, kind="ExternalInput")
    w2_d = nc.dram_tensor("w2", [HID, HID2], f32, kind="ExternalInput")
    t1_d = nc.dram_tensor("t1", [HID, 1], f32, kind="ExternalInput")
    t2_d = nc.dram_tensor("t2", [HID2, 1], f32, kind="ExternalInput")
    fcw_d = nc.dram_tensor("fcw", [HID2, 1], f32, kind="ExternalInput")
    y_d = nc.dram_tensor("y", [P, TILES], f32, kind="ExternalOutput")

    with tile.TileContext(nc) as tc:
        with (
            tc.tile_pool(name="const", bufs=1) as cpool,
            tc.tile_pool(name="upart", bufs=1) as upool,
            tc.tile_pool(name="g1", bufs=2) as g1pool,
            tc.tile_pool(name="g2", bufs=2) as g2pool,
            tc.tile_pool(name="sel", bufs=8) as selpool,
            tc.tile_pool(name="work", bufs=6) as wpool,
            tc.tile_pool(name="pacc", bufs=3, space="PSUM") as pacc,
            tc.tile_pool(name="pmm", bufs=2, space="PSUM") as pmm,
            tc.tile_pool(name="ptr", bufs=2, space="PSUM") as ptr,
            tc.tile_pool(name="dram", bufs=1, space="DRAM") as dpool,
        ):
            nc.gpsimd.load_library(mlp)

            # ---- tensors needed for the dense stage first ----
            dinv_t = cpool.tile([P, TILES], f32)
            nc.sync.dma_start(out=dinv_t[:], in_=dinv_d[:])
            w1_t = cpool.tile([IN_CH, HID], bf)
            nc.sync.dma_start(out=w1_t[:], in_=w1_d[:])
            xfull = cpool.tile([P, SPC], bf)
            nc.sync.dma_start(out=xfull[:, :SZA], in_=xT_d[:, :SZA])
            nc.sync.dma_start(out=xfull[:, SZA:], in_=xT_d[:, SZA:])

            tab1_t = upool.tile([P, TILES, HID], bf, tag="tab1")
            tab2_t = upool.tile([P, TILES, HID2], bf, tag="tab2")
            zsp_t = upool.tile([P, TILES, HID], f32, tag="zsp")
            out_t = upool.tile([P, TILES], f32, tag="out")

            ag1A = dpool.tile([SZA, HID], bf, name="ag1A")
            ag1B = dpool.tile([SZB, HID], bf, name="ag1B")
            s1A = dpool.tile([NLA1, 2 * HID], bf, addr_space="Shared",
                             name="s1A")
            s1B = dpool.tile([NLB1, 2 * HID], bf, addr_space="Shared",
                             name="s1B")
            ag2A = dpool.tile([SZA, HID2], bf, name="ag2A")
            ag2B = dpool.tile([SZB, HID2], bf, name="ag2B")
            s2_tab = dpool.tile([NL2, 4 * HID2], bf, name="s2tab")

            # ---- L1 dense: tab1 = dinv * (x @ W1'), half A then B ----
            def dense(t0, t1r):
                for t in range(t0, t1r):
                    pm = pmm.tile([P, HID], f32, space="PSUM", tag="pm")
                    nc.tensor.matmul(out=pm[:], lhsT=xfull[:, t * P:(t + 1) * P],
                                     rhs=w1_t[:], start=True, stop=True)
                    nc.scalar.activation(out=tab1_t[:, t, :], in_=pm[:],
                                         func=AF.Copy, scale=dinv_t[:, t:t + 1])

            dense(0, TA)
            nc.sync.dma_start(
                out=ag1A[:].rearrange("(t p) w -> p t w", p=P),
                in_=tab1_t[:, :TA, :])
            nc.gpsimd.collective_compute(
                "AllGather", mybir.AluOpType.bypass,
                replica_groups=[list(range(NCORES))],
                ins=[ag1A[:]], outs=[s1A[:]])
            dense(TA, TILES)
            nc.sync.dma_start(
                out=ag1B[:].rearrange("(t p) w -> p t w", p=P),
                in_=tab1_t[:, TA:, :])
            nc.gpsimd.collective_compute(
                "AllGather", mybir.AluOpType.bypass,
                replica_groups=[list(range(NCORES))],
                ins=[ag1B[:]], outs=[s1B[:]])

            # ---- remaining constants (overlap with AG1A/AG1B) ----
            idx1_t = [[cpool.tile([P, w1tot[a][h]], i16, name=f"idx1t{a}{h}")
                       for h in range(2)] for a in range(2)]
            idx2_t = [cpool.tile([P, w2tot[k]], i16, name=f"idx2t{k}")
                      for k in range(4)]
            for a in range(2):
                for h in range(2):
                    nc.sync.dma_start(out=idx1_t[a][h][:], in_=idx1_d[a][h][:])
            for k in range(4):
                nc.sync.dma_start(out=idx2_t[k][:], in_=idx2_d[k][:])
            dest1_t = [cpool.tile([P, NCH1[h]], bf, name=f"dest1t{h}")
                       for h in range(2)]
            for h in range(2):
                nc.sync.dma_start(out=dest1_t[h][:], in_=dest1_d[h][:])
            dest2_t = cpool.tile([P, NCH2], bf)
            nc.sync.dma_start(out=dest2_t[:], in_=dest2_d[:])
            w2_t = cpool.tile([HID, HID2], f32)
            nc.sync.dma_start(out=w2_t[:], in_=w2_d[:])
            t1_t = cpool.tile([HID, 1], f32)
            nc.sync.dma_start(out=t1_t[:], in_=t1_d[:])
            t2_t = cpool.tile([HID2, 1], f32)
            nc.sync.dma_start(out=t2_t[:], in_=t2_d[:])
            fcw_t = cpool.tile([HID2, 1], f32)
            nc.sync.dma_start(out=fcw_t[:], in_=fcw_d[:])

            identf = cpool.tile([P, P], f32)
            make_identity(nc, identf[:])
            identb = cpool.tile([P, P], bf)
            nc.vector.tensor_copy(out=identb[:], in_=identf[:])
            # iota2[p, j, c] = j  (lane on middle axis, chunk innermost)
            iota_i = cpool.tile([P, P * NCHM], mybir.dt.int32)
            nc.gpsimd.iota(iota_i[:], pattern=[[1, P], [0, NCHM]], base=0,
                           channel_multiplier=0)
            iota_b = cpool.tile([P, P * NCHM], bf)
            nc.vector.tensor_copy(out=iota_b[:], in_=iota_i[:])
            iota_r = iota_b[:].rearrange("p (j c) -> p j c", c=NCHM)

            def tab_ap(tab, nlines, sub_off, elem):
                return bassm.AP(tensor=tab[:].tensor, offset=sub_off,
                                ap=[[2 * HID, nlines], [1, elem]])

            def sel_build(dest_t, dcol, nch_t):
                sel = selpool.tile([P, P, NCHM], bf, tag="sel")
                nc.vector.tensor_tensor(
                    out=sel[:, :, :nch_t],
                    in0=dest_t[:, None, dcol:dcol + nch_t]
                        .to_broadcast([P, P, nch_t]),
                    in1=iota_r[:, :, :nch_t],
                    op=mybir.AluOpType.is_equal)
                return sel

            # ---- generic scatter pass ----
            def scatter(groups, tab_of_cls, nlines_of_cls, elem, ncls, cbud,
                        idx_t, dest_t, ga_pool, ga_tag, finish, checkpoints=()):
                goff = [0] * ncls
                dcol = 0
                gmax = max(sum(cbud[t][k] for t in range(g0, g0 + gn))
                           for g0, gn in _spans(groups)
                           for k in range(ncls))
                t0 = 0
                for gn in groups:
                    gas = []
                    for k in range(ncls):
                        gw = sum(cbud[t][k] for t in range(t0, t0 + gn))
                        ga = ga_pool.tile([P, gmax, elem], bf,
                                          tag=f"{ga_tag}_{k}")
                        ni = gw * P
                        if ni:
                            _dma_gather_raw(
                                nc.gpsimd, nc, ga[:, :gw, :],
                                tab_ap(tab_of_cls(k), nlines_of_cls(k),
                                       (k % 2) * elem if elem == HID else k * elem,
                                       elem),
                                idx_t[k][:, goff[k]:goff[k] + ni // 16],
                                ni, elem, 2 * HID, single_packet=False)
                        goff[k] += ni // 16
                        gas.append(ga)
                    coff = [0] * ncls
                    for t in range(t0, t0 + gn):
                        nch_t = sum(cbud[t])
                        acc = pacc.tile([P, elem], f32, space="PSUM", tag="acc")
                        if nch_t:
                            sel = sel_build(dest_t, dcol, nch_t)
                        cc = 0
                        for k in range(ncls):
                            for i in range(cbud[t][k]):
                                nc.tensor.matmul(
                                    out=acc[:], lhsT=sel[:, :, cc],
                                    rhs=gas[k][:, coff[k] + i, :],
                                    start=(cc == 0), stop=False)
                                cc += 1
                            coff[k] += cbud[t][k]
                        dcol += nch_t
                        finish(t, acc, cc == 0)
                        if t + 1 in checkpoints:
                            checkpoints[t + 1]()
                    t0 += gn

            def _spans(groups):
                t0 = 0
                for gn in groups:
                    yield t0, gn
                    t0 += gn

            # ---- L1 pass A: accumulate A-half sources + self, spill ----
            def finA(t, acc, empty):
                nc.tensor.matmul(out=acc[:], lhsT=identb[:],
                                 rhs=tab1_t[:, t, :], start=empty, stop=True)
                nc.scalar.activation(out=zsp_t[:, t, :], in_=acc[:],
                                     func=AF.Copy)

            scatter(GRP1, lambda k: s1A, lambda k: NLA1, HID, 2,
                    [[c1h[t][0][0], c1h[t][1][0]] for t in range(TILES)],
                    [idx1_t[0][0], idx1_t[1][0]], dest1_t[0][:], g1pool, "ga1",
                    finA)

            # ---- L1 pass B: B-half sources + spill re-add, post ----
            def post1(t, acc, empty):
                nc.tensor.matmul(out=acc[:], lhsT=identf[:],
                                 rhs=zsp_t[:, t, :], start=empty, stop=True)
                z = wpool.tile([P, HID], f32, tag="z1")
                nc.scalar.activation(out=z[:], in_=acc[:], func=AF.Copy,
                                     scale=dinv_t[:, t:t + 1])
                zt = ptr.tile([HID, P], f32, space="PSUM", tag="zt")
                nc.tensor.transpose(out=zt[:], in_=z[:], identity=identf[:])
                h2 = wpool.tile([HID, P], f32, tag="h2T")
                nc.scalar.activation(out=h2[:], in_=zt[:], func=AF.Relu,
                                     bias=t1_t[:])
                p2 = pmm.tile([P, HID2], f32, space="PSUM", tag="pm")
                nc.tensor.matmul(out=p2[:], lhsT=h2[:], rhs=w2_t[:],
                                 start=True, stop=True)
                nc.scalar.activation(out=tab2_t[:, t, :], in_=p2[:],
                                     func=AF.Copy, scale=dinv_t[:, t:t + 1])

            def issue_ag2A():
                nc.sync.dma_start(
                    out=ag2A[:].rearrange("(t p) w -> p t w", p=P),
                    in_=tab2_t[:, :TA, :])
                nc.gpsimd.collective_compute(
                    "AllGather", mybir.AluOpType.bypass,
                    replica_groups=[list(range(NCORES))],
                    ins=[ag2A[:]], outs=[bassm.AP(tensor=s2_tab[:].tensor,
                                                  offset=0,
                                                  ap=[[4 * HID2, NL2A],
                                                      [1, 4 * HID2]])])

            scatter(GRP1B, lambda k: s1B, lambda k: NLB1, HID, 2,
                    [[c1h[t][0][1], c1h[t][1][1]] for t in range(TILES)],
                    [idx1_t[0][1], idx1_t[1][1]], dest1_t[1][:], g1pool, "ga1",
                    post1, checkpoints={TA: issue_ag2A})

            nc.sync.dma_start(
                out=ag2B[:].rearrange("(t p) w -> p t w", p=P),
                in_=tab2_t[:, TA:, :])
            nc.gpsimd.collective_compute(
                "AllGather", mybir.AluOpType.bypass,
                replica_groups=[list(range(NCORES))],
                ins=[ag2B[:]], outs=[bassm.AP(tensor=s2_tab[:].tensor,
                                              offset=NL2A * 4 * HID2,
                                              ap=[[4 * HID2, NL2 - NL2A],
                                                  [1, 4 * HID2]])])

            # ---- L2 scatter (single pass over combined table) ----
            def post2(t, acc, empty):
                nc.tensor.matmul(out=acc[:], lhsT=identb[:],
                                 rhs=tab2_t[:, t, :], start=empty, stop=True)
                z = wpool.tile([P, HID2], f32, tag="z2")
                nc.scalar.activation(out=z[:], in_=acc[:], func=AF.Copy,
                                     scale=dinv_t[:, t:t + 1])
                zt = ptr.tile([HID2, P], f32, space="PSUM", tag="zt")
                nc.tensor.transpose(out=zt[:], in_=z[:], identity=identf[:])
                h3 = wpool.tile([HID2, P], f32, tag="h3T")
                nc.scalar.activation(out=h3[:], in_=zt[:], func=AF.Relu,
                                     bias=t2_t[:])
                py = pmm.tile([P, 1], f32, space="PSUM", tag="pm")
                nc.tensor.matmul(out=py[:], lhsT=h3[:], rhs=fcw_t[:],
                                 start=True, stop=True)
                nc.scalar.activation(out=out_t[:, t:t + 1], in_=py[:],
                                     func=AF.Copy)

            scatter(GRP1, lambda k: s2_tab, lambda k: NL2, HID2, 4, c2,
                    idx2_t, dest2_t[:], g2pool, "ga2", post2)

            nc.sync.dma_start(out=y_d[:], in_=out_t[:])

    nc.compile()
    return nc


# ----------------------------------------------------------------------
# entry point
# ----------------------------------------------------------------------
def prepare(inputs):
    inputs = {k: np.asarray(v) for k, v in inputs.items()}
    cores, consts = host_prep(**inputs)
    nc = build_bass(consts["c1h"], consts["c2"])

    t1 = consts["T1"].reshape(HID, 1).astype(np.float32)
    t2 = consts["T2"].reshape(HID2, 1).astype(np.float32)
    fcw = consts["fcW"].reshape(HID2, 1).astype(np.float32)

    in_maps = []
    for c in range(NCORES):
        m = {
            "xT": cores[c]["xT"].astype(BF16),
            "dest2": cores[c]["dest2"],
            "dinv": cores[c]["dinv"],
            "w1": consts["W1p"].astype(BF16),
            "w2": consts["W2p"].astype(np.float32),
            "t1": t1,
            "t2": t2,
            "fcw": fcw,
        }
        for h in range(2):
            m[f"dest1_{h}"] = cores[c]["dest1"][h]
            for a in range(2):
                m[f"idx1_{a}_{h}"] = cores[c]["idx1"][a][h]
        for k in range(4):
            m[f"idx2_{k}"] = cores[c]["idx2"][k]
        in_maps.append(m)
    return nc, in_maps, consts | {"cores": cores}


def execute(nc, in_maps):
    from concourse.bass_utils import run_bass_kernel_spmd
    return run_bass_kernel_spmd(nc, in_maps, core_ids=list(range(NCORES)))


def unshard(res, consts):
    y = np.zeros((N_NODES, 1), np.float32)
    fcb = consts["fcb"]
    for c in range(NCORES):
        nodes = consts["cores"][c]["nodes"]
        occ = nodes >= 0
        vals = res.results[c]["y"].T.reshape(-1) + fcb
        y[nodes[occ], 0] = vals[occ]
    return y


def kernel(**inputs):
    nc, in_maps, consts = prepare(inputs)
    res = execute(nc, in_maps)
    return unshard(res, consts)


# revision 28
# speedup vs baseline: 1.8512x; 1.1146x over previous
"""Distributed 2-layer GCN (BangaloreGCN) on 8 Trainium2 NeuronCores.

v3 strategy (node/graph-parallel):
  * Nodes packed into 8*49 destination bins of 128 lanes (LPT on
    in-degree).  GCN refactored so message passing is gather + one-hot
    matmul segment-sum: out = dinv*(A @ (dinv*h)) + dinv^2*h, BN scale
    folded into W, biases applied channel-major after a PE transpose.
  * Layer tables are densely packed (2 or 4 node rows per 256B gather
    line) and replicated with AllGather collectives split into two
    source-halves (tile slots 0-23 / 24-48) so the second collective
    overlaps compute:
      - L1: AG1A starts right after the first 24 dense tiles; the L1
        scatter is two passes (A-half sources while AG1B transfers,
        then B-half sources + PSUM-spill re-add).
      - L2: W2 applied before the collective (32-wide table); AG2A is
        issued mid-way through the L1 B-pass, one scatter pass.
  * dma_gather streams per source-class (slot parity) with per-tile
    compile-time chunk budgets (max over cores); greedy host-side
    class balancing.  Sel one-hot masks are built chunk-innermost so
    the DVE 2x perf mode applies.
"""

import sys

sys.path.insert(0, "/opt/trn_rl_repo")

import heapq

import ml_dtypes
import numpy as np

BF16 = ml_dtypes.bfloat16

# ---- problem constants (hardcoded per contest contract) ----
N_NODES = 50000
IN_CH = 128
HID = 64
HID2 = 32
BN_EPS = 1e-5

NCORES = 8
P = 128
TILES = 49                 # dest tiles per core
SPC = TILES * P            # slots per core (6272)
NSLOT = NCORES * SPC       # 50176
NBINS = NCORES * TILES     # 392
TA = 28                    # tiles in half A
TB = TILES - TA            # 25
SZA = TA * P               # 3072 slots per core in half A
SZB = TB * P               # 3200
NLA1 = NCORES * SZA // 2   # 12288 L1 lines in table A
NLB1 = NCORES * SZB // 2   # 12800
NL2A = NCORES * SZA // 4   # L2 lines in table A
NL2B = NCORES * SZB // 4
PAD_DEST = 200.0


# ----------------------------------------------------------------------
# host-side preparation
# ----------------------------------------------------------------------
def _pack_nodes_bins(deg_in, n):
    """LPT-pack nodes into NBINS bins of <=128 by in-degree."""
    order = np.argsort(-deg_in, kind="stable")
    heap = [(0, b) for b in range(NBINS)]
    heapq.heapify(heap)
    counts = np.zeros(NBINS, np.int32)
    binof = np.empty(n, np.int32)
    for v in order:
        load, b = heapq.heappop(heap)
        binof[v] = b
        counts[b] += 1
        if counts[b] < P:
            heapq.heappush(heap, (load + int(deg_in[v]), b))
    return binof


def _assign_classes(row, col_bin, out_deg, binof, h_node, n):
    """Greedy mod-4 class per node, balancing per-(dest bin, class,
    source-half) edge counts subject to 32 lanes per class per bin."""
    order_e = np.argsort(row, kind="stable")
    s_sorted = row[order_e]
    starts = np.searchsorted(s_sorted, np.arange(n))
    ends = np.searchsorted(s_sorted, np.arange(n) + 1)
    dbins_sorted = col_bin[order_e]
    cnt = np.zeros((NBINS, 4, 2), np.int64)
    cap = np.full((NBINS, 4), P // 4, np.int32)
    cls = np.empty(n, np.int8)
    for v in np.argsort(-out_deg, kind="stable"):
        b = binof[v]
        h = h_node[v]
        db = dbins_sorted[starts[v]:ends[v]]
        if len(db):
            sc = cnt[db, :, h].sum(axis=0)
        else:
            sc = np.zeros(4, np.int64)
        sc = np.where(cap[b] > 0, sc, 1 << 40)
        c = int(np.argmin(sc))
        cls[v] = c
        cap[b, c] -= 1
        if len(db):
            np.add.at(cnt, (db, c, h), 1)
    return cls


def _wrap_idx(arr):
    ni = arr.shape[0]
    blk = arr.reshape(ni // 16, 16).T.astype(np.int16)
    return np.tile(blk, (8, 1))


def host_prep(x, edge_index, W1, b1, W2, b2, fcW, fcb,
              g1, be1, rm1, rv1, g2, be2, rm2, rv2):
    n = x.shape[0]
    row = np.asarray(edge_index[0], np.int64)
    col = np.asarray(edge_index[1], np.int64)

    deg = np.bincount(col, minlength=n).astype(np.float32) + 1.0
    dinv = (1.0 / np.sqrt(deg)).astype(np.float32)
    deg_in = np.bincount(col, minlength=n)
    deg_out = np.bincount(row, minlength=n)

    binof = _pack_nodes_bins(deg_in, n)

    # per-core: sort own bins desc by in-degree -> tile slots
    bin_in = np.bincount(binof[col], minlength=NBINS)
    tslot_of_bin = np.empty(NBINS, np.int64)
    for c in range(NCORES):
        bins = np.arange(c * TILES, (c + 1) * TILES)
        order_b = bins[np.argsort(-bin_in[bins], kind="stable")]
        tslot_of_bin[order_b] = np.arange(TILES)
    rbin = (np.arange(NBINS) // TILES) * TILES + tslot_of_bin

    h_node = (tslot_of_bin[binof] >= TA).astype(np.int8)   # source half per node
    cls4 = _assign_classes(row, rbin[binof[col]], deg_out, binof, h_node, n)

    # lanes: class c gets lanes {c, c+4, ...} within its bin
    lane = np.empty(n, np.int64)
    key = binof.astype(np.int64) * 4 + cls4
    order = np.argsort(key, kind="stable")
    uniq, first = np.unique(key[order], return_index=True)
    rank = np.arange(n) - np.repeat(first, np.diff(np.append(first, n)))
    lane[order] = cls4[order] + 4 * rank
    assert lane.max() < P

    slot_of_node = rbin[binof] * P + lane          # table slot (core-major)
    node_of_slot = np.full(NSLOT, -1, np.int64)
    node_of_slot[slot_of_node] = np.arange(n)

    src_slot = slot_of_node[row]
    dst_slot = slot_of_node[col]
    dlane = dst_slot % P
    scls = (src_slot % 4).astype(np.int64)

    core_of_edge = (dst_slot // P) // TILES
    tslot = (dst_slot // P) % TILES

    # source half + half-local slot / lines
    s_core = src_slot // SPC
    s_t = (src_slot % SPC) // P
    s_lane = src_slot % P
    s_h = (s_t >= TA).astype(np.int64)
    sH = np.where(s_h == 0,
                  s_core * SZA + s_t * P + s_lane,
                  s_core * SZB + (s_t - TA) * P + s_lane)
    line1 = sH >> 1                                # within half table
    line2 = sH >> 2                                # within half table

    # budgets
    cnt1 = np.zeros((NCORES, TILES, 2, 2), np.int64)   # [core, t, cls2, half]
    np.add.at(cnt1, (core_of_edge, tslot, scls % 2, s_h), 1)
    c1h = -(-cnt1.max(axis=0) // P)                    # [TILES, 2, 2]
    cnt4 = np.zeros((NCORES, TILES, 4, 2), np.int64)
    np.add.at(cnt4, (core_of_edge, tslot, scls, s_h), 1)
    c2h = -(-cnt4.max(axis=0) // P)                    # [TILES, 4, 2]

    # sort edges by (core, tslot, cls4, half)
    ekey = (((core_of_edge * TILES + tslot) * 4 + scls) * 2 + s_h)
    eorder = np.argsort(ekey, kind="stable")
    e_line1 = line1[eorder]
    e_line2 = line2[eorder]
    e_dlane = dlane[eorder]
    e_key = ekey[eorder]
    bounds = np.searchsorted(e_key, np.arange(NCORES * TILES * 8 + 1))

    S1c = (g1 / np.sqrt(rv1 + BN_EPS)).astype(np.float32)
    T1 = ((b1 - rm1) * S1c + be1).astype(np.float32)
    S2c = (g2 / np.sqrt(rv2 + BN_EPS)).astype(np.float32)
    T2 = ((b2 - rm2) * S2c + be2).astype(np.float32)
    W1p = (W1 * S1c[None, :]).astype(np.float32)
    W2p = (W2 * S2c[None, :]).astype(np.float32)

    NCH1 = [int(c1h[:, :, h].sum()) for h in range(2)]   # dest-img cols per half
    NCH2 = [int(c2h[:, :, h].sum()) for h in range(2)]

    cores = []
    for c in range(NCORES):
        idx1 = [[np.zeros(int(c1h[:, a, h].sum()) * P, np.int64)
                 for h in range(2)] for a in range(2)]
        idx2 = [[np.zeros(int(c2h[:, k, h].sum()) * P, np.int64)
                 for h in range(2)] for k in range(4)]
        dest1 = [np.full((NCH1[h], P), PAD_DEST, np.float32) for h in range(2)]
        dest2 = [np.full((NCH2[h], P), PAD_DEST, np.float32) for h in range(2)]
        off1 = [[0, 0], [0, 0]]
        off2 = [[0, 0], [0, 0], [0, 0], [0, 0]]
        col1 = [0, 0]
        col2 = [0, 0]

        def sl(t, k, h):
            i = ((c * TILES + t) * 4 + k) * 2 + h
            return bounds[i], bounds[i + 1]

        for t in range(TILES):
            for h in range(2):
                for k in range(4):
                    lo, hi = sl(t, k, h)
                    li = e_line2[lo:hi]
                    dl = e_dlane[lo:hi]
                    cap = int(c2h[t, k, h]) * P
                    assert len(li) <= cap
                    idx2[k][h][off2[k][h]:off2[k][h] + len(li)] = li
                    d = dest2[h][col2[h]:col2[h] + c2h[t, k, h]].reshape(-1)
                    d[:len(li)] = dl
                    off2[k][h] += cap
                    col2[h] += int(c2h[t, k, h])
            for h in range(2):
                for a in range(2):
                    parts = []
                    for k in (a, a + 2):
                        lo, hi = sl(t, k, h)
                        parts.append((e_line1[lo:hi], e_dlane[lo:hi]))
                    li = np.concatenate([p[0] for p in parts])
                    dl = np.concatenate([p[1] for p in parts])
                    cap = int(c1h[t, a, h]) * P
                    assert len(li) <= cap
                    idx1[a][h][off1[a][h]:off1[a][h] + len(li)] = li
                    d = dest1[h][col1[h]:col1[h] + c1h[t, a, h]].reshape(-1)
                    d[:len(li)] = dl
                    off1[a][h] += cap
                    col1[h] += int(c1h[t, a, h])

        nodes = node_of_slot[c * SPC:(c + 1) * SPC]
        occ = nodes >= 0
        xs = np.zeros((SPC, IN_CH), np.float32)
        xs[occ] = x[nodes[occ]]
        dv = np.zeros(SPC, np.float32)
        dv[occ] = dinv[nodes[occ]]

        cores.append(dict(
            idx1=[[_wrap_idx(idx1[a][h]) for h in range(2)] for a in range(2)],
            idx2=[[_wrap_idx(idx2[k][h]) for h in range(2)] for k in range(4)],
            dest1=[dest1[h].T.astype(BF16).copy() for h in range(2)],
            dest2=[dest2[h].T.astype(BF16).copy() for h in range(2)],
            xT=np.ascontiguousarray(xs.T),
            dinv=np.ascontiguousarray(dv.reshape(TILES, P).T),
            nodes=nodes,
        ))

    consts = dict(W1p=W1p, W2p=W2p, T1=T1, T2=T2,
                  fcW=np.asarray(fcW, np.float32),
                  fcb=float(np.asarray(fcb).reshape(-1)[0]),
                  c1h=c1h, c2h=c2h)
    return cores, consts


# ----------------------------------------------------------------------
# device program
# ----------------------------------------------------------------------
def _dma_gather_raw(gp, bassmod, out_ap, in_ap, idxs_ap, num_idxs, elem_size,
                    elem_step, single_packet=True, queue_num=0):
    """bass.dma_gather with elem_size_bytes below 256B allowed (stride must
    still be a multiple of 256B)."""
    import concourse.mybir as mybir
    from concourse import ap_utils
    from concourse.bass import MemorySpace, exact_div, round_up_to_multiple

    assert idxs_ap.dtype == mybir.dt.int16
    assert in_ap.dtype == out_ap.dtype
    assert in_ap.space == MemorySpace.DRAM
    assert idxs_ap.space == MemorySpace.SBUF and out_ap.space == MemorySpace.SBUF
    assert ap_utils.ap_is_contiguous(out_ap.ap[1:])
    assert ap_utils.ap_is_contiguous(idxs_ap.ap[1:])
    assert in_ap.ap[-1][1] == out_ap.ap[-1][1] == elem_size
    assert out_ap.ap[0][1] * out_ap.ap[1][1] == round_up_to_multiple(num_idxs, 128)
    assert in_ap.ap[0][0] == elem_step
    stride_bytes_256 = exact_div(elem_step * mybir.dt.size(in_ap.dtype), 256)
    assert stride_bytes_256 < 256
    return gp.add_instruction(
        mybir.InstDMAGatherAnt(
            name=bassmod.get_next_instruction_name(),
            ins=[*gp.lower_ap_dma(in_ap, for_custom_bir_dma=True),
                 gp.lower_ap(idxs_ap),
                 gp.lower_val_access(gp.to_reg(num_idxs))],
            outs=[gp.lower_ap(out_ap)],
            transpose=False,
            num_idxs=num_idxs,
            elem_size=elem_size,
            stride_bytes_256=stride_bytes_256,
            gen_mode=0,
            single_packet=single_packet,
            queue_num=queue_num,
            sbuf_tokens_per_rank=0,
            sbuf_free_dim_per_rank=0,
            sbuf_free_dim_pad_per_rank=0,
            sbuf_byte_offset=0,
        ))


GRP1 = [10, 10, 10, 10, 9]     # gather groups (tiles per group)
GRP1B = GRP1
GRP2 = [13, 12, 12, 12]        # L2 pass groups


def build_bass(c1h, c2h):
    import concourse.bacc as bacc
    import concourse.bass as bassm
    import concourse.mybir as mybir
    import concourse.tile as tile
    from concourse.library_config import mlp
    from concourse.masks import make_identity

    f32 = mybir.dt.float32
    bf = mybir.dt.bfloat16
    i16 = mybir.dt.int16
    AF = mybir.ActivationFunctionType

    c1h = [[[int(c1h[t][a][h]) for h in range(2)] for a in range(2)]
           for t in range(TILES)]
    c2h = [[[int(c2h[t][k][h]) for h in range(2)] for k in range(4)]
           for t in range(TILES)]
    nch1 = [[sum(c1h[t][a][h] for a in range(2)) for h in range(2)]
            for t in range(TILES)]
    nch2 = [[sum(c2h[t][k][h] for k in range(4)) for h in range(2)]
            for t in range(TILES)]
    NCH1 = [sum(nch1[t][h] for t in range(TILES)) for h in range(2)]
    NCH2 = [sum(nch2[t][h] for t in range(TILES)) for h in range(2)]
    NCHM = max(max(max(nch1[t]) for t in range(TILES)),
               max(max(nch2[t]) for t in range(TILES)))
    w1tot = [[sum(c1h[t][a][h] for t in range(TILES)) * 8 for h in range(2)]
             for a in range(2)]
    w2tot = [[sum(c2h[t][k][h] for t in range(TILES)) * 8 for h in range(2)]
             for k in range(4)]

    nc = bacc.Bacc("TRN2", target_bir_lowering=False)
    xT_d = nc.dram_tensor("xT", [P, SPC], bf, kind="ExternalInput")
    idx1_d = [[nc.dram_tensor(f"idx1_{a}_{h}", [P, w1tot[a][h]], i16,
                              kind="ExternalInput") for h in range(2)]
              for a in range(2)]
    idx2_d = [[nc.dram_tensor(f"idx2_{k}_{h}", [P, w2tot[k][h]], i16,
                              kind="ExternalInput") for h in range(2)]
              for k in range(4)]
    dest1_d = [nc.dram_tensor(f"dest1_{h}", [P, NCH1[h]], bf,
                              kind="ExternalInput") for h in range(2)]
    dest2_d = [nc.dram_tensor(f"dest2_{h}", [P, NCH2[h]], bf,
                              kind="ExternalInput") for h in range(2)]
    dinv_d = nc.dram_tensor("dinv", [P, TILES], f32, kind="ExternalInput")
    w1_d = nc.dram_tensor("w1", [IN_CH, HID], bf, kind="ExternalInput")
    w2_d = nc.dram_tensor("w2", [HID, HID2], f32, kind="ExternalInput")
    t1_d = nc.dram_tensor("t1", [HID, 1], f32, kind="ExternalInput")
    t2_d = nc.dram_tensor("t2", [HID2, 1], f32, kind="ExternalInput")
    fcw_d = nc.dram_tensor("fcw", [HID2, 1], f32, kind="ExternalInput")
    y_d = nc.dram_tensor("y", [P, TILES], f32, kind="ExternalOutput")

    with tile.TileContext(nc) as tc:
        with (
            tc.tile_pool(name="const", bufs=1) as cpool,
            tc.tile_pool(name="upart", bufs=1) as upool,
            tc.tile_pool(name="g1", bufs=2) as g1pool,
            tc.tile_pool(name="g2", bufs=2) as g2pool,
            tc.tile_pool(name="sel", bufs=8) as selpool,
            tc.tile_pool(name="work", bufs=6) as wpool,
            tc.tile_pool(name="pacc", bufs=3, space="PSUM") as pacc,
            tc.tile_pool(name="pmm", bufs=2, space="PSUM") as pmm,
            tc.tile_pool(name="ptr", bufs=2, space="PSUM") as ptr,
            tc.tile_pool(name="dram", bufs=1, space="DRAM") as dpool,
        ):
            nc.gpsimd.load_library(mlp)

            # ---- tensors needed for the dense stage first ----
            dinv_t = cpool.tile([P, TILES], f32)
            nc.sync.dma_start(out=dinv_t[:], in_=dinv_d[:])
            w1_t = cpool.tile([IN_CH, HID], bf)
            nc.sync.dma_start(out=w1_t[:], in_=w1_d[:])
            xfull = cpool.tile([P, SPC], bf)
            nc.sync.dma_start(out=xfull[:, :SZA], in_=xT_d[:, :SZA])
            nc.sync.dma_start(out=xfull[:, SZA:], in_=xT_d[:, SZA:])

            tab1_t = upool.tile([P, TILES, HID], bf, tag="tab1")
            tab2_t = upool.tile([P, TILES, HID2], bf, tag="tab2")
            zsp_t = upool.tile([P, TILES, HID], f32, tag="zsp")
            zsp2_t = upool.tile([P, TILES, HID2], f32, tag="zsp2")
            out_t = upool.tile([P, TILES], f32, tag="out")

            ag1A = dpool.tile([SZA, HID], bf, name="ag1A")
            ag1B = dpool.tile([SZB, HID], bf, name="ag1B")
            s1A = dpool.tile([NLA1, 2 * HID], bf, addr_space="Shared",
                             name="s1A")
            s1B = dpool.tile([NLB1, 2 * HID], bf, addr_space="Shared",
                             name="s1B")
            ag2A = dpool.tile([SZA, HID2], bf, name="ag2A")
            ag2B = dpool.tile([SZB, HID2], bf, name="ag2B")
            s2A = dpool.tile([NL2A, 4 * HID2], bf, addr_space="Shared",
                             name="s2A")
            s2B = dpool.tile([NL2B, 4 * HID2], bf, addr_space="Shared",
                             name="s2B")

            # ---- L1 dense: tab1 = dinv * (x @ W1'), half A then B ----
            def dense(t0, t1r):
                for t in range(t0, t1r):
                    pm = pmm.tile([P, HID], f32, space="PSUM", tag="pm")
                    nc.tensor.matmul(out=pm[:], lhsT=xfull[:, t * P:(t + 1) * P],
                                     rhs=w1_t[:], start=True, stop=True)
                    nc.scalar.activation(out=tab1_t[:, t, :], in_=pm[:],
                                         func=AF.Copy, scale=dinv_t[:, t:t + 1])

            dense(0, TA)
            nc.sync.dma_start(
                out=ag1A[:].rearrange("(t p) w -> p t w", p=P),
                in_=tab1_t[:, :TA, :])
            nc.gpsimd.collective_compute(
                "AllGather", mybir.AluOpType.bypass,
                replica_groups=[list(range(NCORES))],
                ins=[ag1A[:]], outs=[s1A[:]])
            dense(TA, TILES)
            nc.sync.dma_start(
                out=ag1B[:].rearrange("(t p) w -> p t w", p=P),
                in_=tab1_t[:, TA:, :])
            nc.gpsimd.collective_compute(
                "AllGather", mybir.AluOpType.bypass,
                replica_groups=[list(range(NCORES))],
                ins=[ag1B[:]], outs=[s1B[:]])

            # ---- remaining constants (overlap with AG1A/AG1B) ----
            idx1_t = [[cpool.tile([P, w1tot[a][h]], i16, name=f"idx1t{a}{h}")
                       for h in range(2)] for a in range(2)]
            idx2_t = [[cpool.tile([P, w2tot[k][h]], i16, name=f"idx2t{k}{h}")
                       for h in range(2)] for k in range(4)]
            for a in range(2):
                for h in range(2):
                    nc.sync.dma_start(out=idx1_t[a][h][:], in_=idx1_d[a][h][:])
            for k in range(4):
                for h in range(2):
                    nc.sync.dma_start(out=idx2_t[k][h][:], in_=idx2_d[k][h][:])
            dest1_t = [cpool.tile([P, NCH1[h]], bf, name=f"dest1t{h}")
                       for h in range(2)]
            for h in range(2):
                nc.sync.dma_start(out=dest1_t[h][:], in_=dest1_d[h][:])
            dest2_t = [cpool.tile([P, NCH2[h]], bf, name=f"dest2t{h}")
                       for h in range(2)]
            for h in range(2):
                nc.sync.dma_start(out=dest2_t[h][:], in_=dest2_d[h][:])
            w2_t = cpool.tile([HID, HID2], f32)
            nc.sync.dma_start(out=w2_t[:], in_=w2_d[:])
            t1_t = cpool.tile([HID, 1], f32)
            nc.sync.dma_start(out=t1_t[:], in_=t1_d[:])
            t2_t = cpool.tile([HID2, 1], f32)
            nc.sync.dma_start(out=t2_t[:], in_=t2_d[:])
            fcw_t = cpool.tile([HID2, 1], f32)
            nc.sync.dma_start(out=fcw_t[:], in_=fcw_d[:])

            identf = cpool.tile([P, P], f32)
            make_identity(nc, identf[:])
            identb = cpool.tile([P, P], bf)
            nc.vector.tensor_copy(out=identb[:], in_=identf[:])
            # iota2[p, j, c] = j  (lane on middle axis, chunk innermost)
            iota_i = cpool.tile([P, P * NCHM], mybir.dt.int32)
            nc.gpsimd.iota(iota_i[:], pattern=[[1, P], [0, NCHM]], base=0,
                           channel_multiplier=0)
            iota_b = cpool.tile([P, P * NCHM], bf)
            nc.vector.tensor_copy(out=iota_b[:], in_=iota_i[:])
            iota_r = iota_b[:].rearrange("p (j c) -> p j c", c=NCHM)

            def tab_ap(tab, nlines, sub_off, elem):
                return bassm.AP(tensor=tab[:].tensor, offset=sub_off,
                                ap=[[2 * HID, nlines], [1, elem]])

            def sel_build(dest_t, dcol, nch_t):
                sel = selpool.tile([P, P, NCHM], bf, tag="sel")
                nc.vector.tensor_tensor(
                    out=sel[:, :, :nch_t],
                    in0=dest_t[:, None, dcol:dcol + nch_t]
                        .to_broadcast([P, P, nch_t]),
                    in1=iota_r[:, :, :nch_t],
                    op=mybir.AluOpType.is_equal)
                return sel

            # ---- generic scatter pass ----
            def scatter(groups, tab_of_cls, nlines_of_cls, elem, ncls, cbud,
                        idx_t, dest_t, ga_pool, ga_tag, finish, checkpoints=()):
                goff = [0] * ncls
                dcol = 0
                gmax = max(sum(cbud[t][k] for t in range(g0, g0 + gn))
                           for g0, gn in _spans(groups)
                           for k in range(ncls))
                t0 = 0
                for gn in groups:
                    gas = []
                    for k in range(ncls):
                        gw = sum(cbud[t][k] for t in range(t0, t0 + gn))
                        ga = ga_pool.tile([P, gmax, elem], bf,
                                          tag=f"{ga_tag}_{k}")
                        ni = gw * P
                        if ni:
                            _dma_gather_raw(
                                nc.gpsimd, nc, ga[:, :gw, :],
                                tab_ap(tab_of_cls(k), nlines_of_cls(k),
                                       (k % 2) * elem if elem == HID else k * elem,
                                       elem),
                                idx_t[k][:, goff[k]:goff[k] + ni // 16],
                                ni, elem, 2 * HID, single_packet=False)
                        goff[k] += ni // 16
                        gas.append(ga)
                    coff = [0] * ncls
                    for t in range(t0, t0 + gn):
                        nch_t = sum(cbud[t])
                        acc = pacc.tile([P, elem], f32, space="PSUM", tag="acc")
                        if nch_t:
                            sel = sel_build(dest_t, dcol, nch_t)
                        cc = 0
                        for k in range(ncls):
                            for i in range(cbud[t][k]):
                                nc.tensor.matmul(
                                    out=acc[:], lhsT=sel[:, :, cc],
                                    rhs=gas[k][:, coff[k] + i, :],
                                    start=(cc == 0), stop=False)
                                cc += 1
                            coff[k] += cbud[t][k]
                        dcol += nch_t
                        finish(t, acc, cc == 0)
                        if t + 1 in checkpoints:
                            checkpoints[t + 1]()
                    t0 += gn

            def _spans(groups):
                t0 = 0
                for gn in groups:
                    yield t0, gn
                    t0 += gn

            # ---- L1 pass A: accumulate A-half sources + self, spill ----
            def finA(t, acc, empty):
                nc.tensor.matmul(out=acc[:], lhsT=identb[:],
                                 rhs=tab1_t[:, t, :], start=empty, stop=True)
                nc.scalar.activation(out=zsp_t[:, t, :], in_=acc[:],
                                     func=AF.Copy)

            scatter(GRP1, lambda k: s1A, lambda k: NLA1, HID, 2,
                    [[c1h[t][0][0], c1h[t][1][0]] for t in range(TILES)],
                    [idx1_t[0][0], idx1_t[1][0]], dest1_t[0][:], g1pool, "ga1",
                    finA)

            # ---- L1 pass B: B-half sources + spill re-add, post ----
            def post1(t, acc, empty):
                nc.tensor.matmul(out=acc[:], lhsT=identf[:],
                                 rhs=zsp_t[:, t, :], start=empty, stop=True)
                z = wpool.tile([P, HID], f32, tag="z1")
                nc.scalar.activation(out=z[:], in_=acc[:], func=AF.Copy,
                                     scale=dinv_t[:, t:t + 1])
                zt = ptr.tile([HID, P], f32, space="PSUM", tag="zt")
                nc.tensor.transpose(out=zt[:], in_=z[:], identity=identf[:])
                h2 = wpool.tile([HID, P], f32, tag="h2T")
                nc.scalar.activation(out=h2[:], in_=zt[:], func=AF.Relu,
                                     bias=t1_t[:])
                p2 = pmm.tile([P, HID2], f32, space="PSUM", tag="pm")
                nc.tensor.matmul(out=p2[:], lhsT=h2[:], rhs=w2_t[:],
                                 start=True, stop=True)
                nc.scalar.activation(out=tab2_t[:, t, :], in_=p2[:],
                                     func=AF.Copy, scale=dinv_t[:, t:t + 1])

            def issue_ag2A():
                nc.sync.dma_start(
                    out=ag2A[:].rearrange("(t p) w -> p t w", p=P),
                    in_=tab2_t[:, :TA, :])
                nc.gpsimd.collective_compute(
                    "AllGather", mybir.AluOpType.bypass,
                    replica_groups=[list(range(NCORES))],
                    ins=[ag2A[:]], outs=[s2A[:]])

            scatter(GRP1B, lambda k: s1B, lambda k: NLB1, HID, 2,
                    [[c1h[t][0][1], c1h[t][1][1]] for t in range(TILES)],
                    [idx1_t[0][1], idx1_t[1][1]], dest1_t[1][:], g1pool, "ga1",
                    post1, checkpoints={TA: issue_ag2A})

            nc.sync.dma_start(
                out=ag2B[:].rearrange("(t p) w -> p t w", p=P),
                in_=tab2_t[:, TA:, :])
            nc.gpsimd.collective_compute(
                "AllGather", mybir.AluOpType.bypass,
                replica_groups=[list(range(NCORES))],
                ins=[ag2B[:]], outs=[s2B[:]])

            # ---- L2 pass A: A-half sources + self, spill ----
            def finA2(t, acc, empty):
                nc.tensor.matmul(out=acc[:], lhsT=identb[:],
                                 rhs=tab2_t[:, t, :], start=empty, stop=True)
                nc.scalar.activation(out=zsp2_t[:, t, :], in_=acc[:],
                                     func=AF.Copy)

            scatter(GRP2, lambda k: s2A, lambda k: NL2A, HID2, 4,
                    [[c2h[t][k][0] for k in range(4)] for t in range(TILES)],
                    [idx2_t[k][0] for k in range(4)], dest2_t[0][:], g2pool,
                    "ga2", finA2)

            # ---- L2 pass B: B-half sources + spill re-add, post ----
            def post2(t, acc, empty):
                nc.tensor.matmul(out=acc[:], lhsT=identf[:],
                                 rhs=zsp2_t[:, t, :], start=empty, stop=True)
                z = wpool.tile([P, HID2], f32, tag="z2")
                nc.scalar.activation(out=z[:], in_=acc[:], func=AF.Copy,
                                     scale=dinv_t[:, t:t + 1])
                zt = ptr.tile([HID2, P], f32, space="PSUM", tag="zt")
                nc.tensor.transpose(out=zt[:], in_=z[:], identity=identf[:])
                h3 = wpool.tile([HID2, P], f32, tag="h3T")
                nc.scalar.activation(out=h3[:], in_=zt[:], func=AF.Relu,
                                     bias=t2_t[:])
                py = pmm.tile([P, 1], f32, space="PSUM", tag="pm")
                nc.tensor.matmul(out=py[:], lhsT=h3[:], rhs=fcw_t[:],
                                 start=True, stop=True)
                nc.scalar.activation(out=out_t[:, t:t + 1], in_=py[:],
                                     func=AF.Copy)

            scatter(GRP2, lambda k: s2B, lambda k: NL2B, HID2, 4,
                    [[c2h[t][k][1] for k in range(4)] for t in range(TILES)],
                    [idx2_t[k][1] for k in range(4)], dest2_t[1][:], g2pool,
                    "ga2", post2)

            nc.sync.dma_start(out=y_d[:], in_=out_t[:])

    nc.compile()
    return nc


# ----------------------------------------------------------------------
# entry point
# ----------------------------------------------------------------------
def prepare(inputs):
    inputs = {k: np.asarray(v) for k, v in inputs.items()}
    cores, consts = host_prep(**inputs)
    nc = build_bass(consts["c1h"], consts["c2h"])

    t1 = consts["T1"].reshape(HID, 1).astype(np.float32)
    t2 = consts["T2"].reshape(HID2, 1).astype(np.float32)
    fcw = consts["fcW"].reshape(HID2, 1).astype(np.float32)

    in_maps = []
    for c in range(NCORES):
        m = {
            "xT": cores[c]["xT"].astype(BF16),
            "dinv": cores[c]["dinv"],
            "w1": consts["W1p"].astype(BF16),
            "w2": consts["W2p"].astype(np.float32),
            "t1": t1,
            "t2": t2,
            "fcw": fcw,
        }
        for h in range(2):
            m[f"dest1_{h}"] = cores[c]["dest1"][h]
            m[f"dest2_{h}"] = cores[c]["dest2"][h]
            for a in range(2):
                m[f"idx1_{a}_{h}"] = cores[c]["idx1"][a][h]
            for k in range(4):
                m[f"idx2_{k}_{h}"] = cores[c]["idx2"][k][h]
        in_maps.append(m)
    return nc, in_maps, consts | {"cores": cores}


def execute(nc, in_maps):
    from concourse.bass_utils import run_bass_kernel_spmd
    return run_bass_kernel_spmd(nc, in_maps, core_ids=list(range(NCORES)))


def unshard(res, consts):
    y = np.zeros((N_NODES, 1), np.float32)
    fcb = consts["fcb"]
    for c in range(NCORES):
        nodes = consts["cores"][c]["nodes"]
        occ = nodes >= 0
        vals = res.results[c]["y"].T.reshape(-1) + fcb
        y[nodes[occ], 0] = vals[occ]
    return y


def kernel(**inputs):
    nc, in_maps, consts = prepare(inputs)
    res = execute(nc, in_maps)
    return unshard(res, consts)
